# revision 2
# baseline (speedup 1.0000x reference)
"""Trainium2 Bass kernel for nn_GTCNN (product-graph GTCNN, 2 layers, K collapsed).

Math (per batch b, x: [M=8192, 32]):
  Adj = s0*I + s1*kron(I_t, As) + s2*kron(At, I_s) + s3*kron(At, As),  T=64, N=128
  h0 = x @ W1 + b1
  h_{l+1} = tanh((Adj @ h_l) @ Heff_l),   Heff_l = sum_k H[l, k]   (einsum collapses k)
  out = h2 @ W2 + b2

Device algorithm per layer (the three mixes commute):
  u = At-mix(z) over t;  Zpre = P(n-mix) z + Q(n-mix) u;  z' = tanh(Zpre @ Heff)
  with P = s0*I + s1*As, Q = s2*I + s3*As folded on host.

Sharding: core c -> (b = c // 4, t-quarter q = c % 4). Layer 1 computed fully per
b (4x redundant, no collectives); layer 2 + output restricted to the 16-t quarter.

Layouts (n = 32*nh + nl, t = 32*c + tl):
  NM  [n, t*32 + h]                      node-on-partition (As / P / Q matmuls)
  FD  [32*nh + h,  t*32 + nl]            feature-on-partition (W1/Heff/W2 matmuls,
                                         block-diag kron(I4, W) stationaries)
  FDT [32*nh + tl, c*1024 + nl*32 + h]   t-on-partition (At matmuls, stationaries
                                         kron(I4, At 32x32 block), PSUM-accum over c)
Layout moves are DVE 32x32 StreamTranspose ops whose in/out access patterns
steer which (block, within-block) geometry is transposed.
"""

import numpy as np

T, NS, B, FIN, HID, FOUT = 64, 128, 2, 32, 32, 16
M = T * NS
NCORES, NQ = 8, 4
TQ = T // NQ  # 16 t's per quarter

_CACHE = {}


def _build_nc():
    from contextlib import ExitStack

    import concourse.mybir as mybir
    import concourse.tile as tile
    from concourse import bacc
    from concourse.bass import ds

    fp = mybir.dt.float32
    AF = mybir.ActivationFunctionType

    nc = bacc.Bacc(
        "TRN2",
        target_bir_lowering=False,
        debug=False,
        enable_asserts=False,
        num_devices=NCORES,
    )

    xb = nc.dram_tensor("xb", [M, FIN], fp, kind="ExternalInput")
    w1i4 = nc.dram_tensor("w1i4", [128, 128], fp, kind="ExternalInput")
    b1t = nc.dram_tensor("b1t", [128, 1], fp, kind="ExternalInput")
    atbd = nc.dram_tensor("atbd", [2, 2, 128, 128], fp, kind="ExternalInput")
    atbq = nc.dram_tensor("atbq", [2, 128, 128], fp, kind="ExternalInput")
    pq = nc.dram_tensor("pq", [2, 128, 128], fp, kind="ExternalInput")
    hi4 = nc.dram_tensor("hi4", [2, 128, 128], fp, kind="ExternalInput")
    w2i4 = nc.dram_tensor("w2i4", [128, 128], fp, kind="ExternalInput")
    b2t = nc.dram_tensor("b2t", [128, 1], fp, kind="ExternalInput")
    outb = nc.dram_tensor("outb", [TQ * NS, FOUT], fp, kind="ExternalOutput")

    C512 = [slice(512 * j, 512 * (j + 1)) for j in range(4)]

    with tile.TileContext(nc) as tc, ExitStack() as ctx:
        const = ctx.enter_context(tc.tile_pool(name="const", bufs=1))
        st = ctx.enter_context(tc.tile_pool(name="st", bufs=1))
        ps = ctx.enter_context(tc.tile_pool(name="ps", bufs=2, space="PSUM"))

        # Core id -> quarter offset register (used for the layer-2 column slice).
        pid = nc.vector.partition_id()
        toff = (pid % NQ) * TQ  # t-offset of this core's quarter

        # ---- constants to SBUF ----
        w1i4_s = const.tile([128, 128], fp, tag="w1i4")
        nc.sync.dma_start(w1i4_s[:], w1i4.ap())
        b1t_s = const.tile([128, 1], fp, tag="b1t")
        nc.sync.dma_start(b1t_s[:], b1t.ap())
        atbd_s = const.tile([128, 4, 128], fp, tag="atbd")
        nc.sync.dma_start(atbd_s[:], atbd.ap().rearrange("a b p c -> p (a b) c"))
        atbq_s = const.tile([128, 2, 128], fp, tag="atbq")
        nc.sync.dma_start(atbq_s[:], atbq.ap().rearrange("a p c -> p a c"))
        pq_s = const.tile([128, 2, 128], fp, tag="pq")
        nc.sync.dma_start(pq_s[:], pq.ap().rearrange("a p c -> p a c"))
        hi4_s = const.tile([128, 2, 128], fp, tag="hi4")
        nc.sync.dma_start(hi4_s[:], hi4.ap().rearrange("a p c -> p a c"))
        w2i4_s = const.tile([128, 128], fp, tag="w2i4")
        nc.sync.dma_start(w2i4_s[:], w2i4.ap())
        b2t_s = const.tile([128, 1], fp, tag="b2t")
        nc.sync.dma_start(b2t_s[:], b2t.ap())
        pmat = pq_s[:, 0, :]
        qmat = pq_s[:, 1, :]

        # ---- x load: NM [n, (t, f)] ----
        x_nm = st.tile([128, 2048], fp, tag="x_nm")
        xv = xb.ap().rearrange("(t n) f -> n t f", n=128)
        x_nm_v = x_nm[:].rearrange("p (t f) -> p t f", f=32)
        for j in range(4):
            nc.sync.dma_start(x_nm_v[:, 16 * j : 16 * (j + 1), :], xv[:, 16 * j : 16 * (j + 1), :])

        # ---- x -> FD ----
        x_fd = st.tile([128, 2048], fp, tag="x_fd")
        for j in range(4):
            nc.vector.transpose(out=x_fd[:, C512[j]], in_=x_nm[:, C512[j]])

        # ---- h0 = x @ W1 + b1 (FD-out) ----
        h0pre = ps.tile([128, 2048], fp, tag="big")
        for j in range(4):
            nc.tensor.matmul(h0pre[:, C512[j]], w1i4_s[:], x_fd[:, C512[j]], start=True, stop=True)
        h0_fd = st.tile([128, 2048], fp, tag="h0_fd")
        for j in range(4):
            nc.scalar.activation(h0_fd[:, C512[j]], h0pre[:, C512[j]], AF.Identity, bias=b1t_s[:, 0:1])

        def t_and_n_mix(z_fd, g_tag, full):
            """From z in FD layout produce (z_nm or None, u contributions).

            Returns (g_fdt tile, z_nm tile) for the full path; layer 2 slices are
            handled by the caller."""
            g = st.tile([128, 2048], fp, tag=g_tag)
            gi = z_fd[:].rearrange("p (c tl nl) -> p c nl tl", c=2, tl=32, nl=32)
            go = g[:].rearrange("p (c nl h) -> p c nl h", c=2, nl=32, h=32)
            for c in range(2):
                for k in range(2):
                    nc.vector.transpose(
                        out=go[:, c, 16 * k : 16 * (k + 1), :], in_=gi[:, c, 16 * k : 16 * (k + 1), :]
                    )
            return g

        # =========================== layer 1 (full) ===========================
        g1 = t_and_n_mix(h0_fd, "g", True)
        z_nm = st.tile([128, 2048], fp, tag="z_nm")
        for j in range(4):
            nc.vector.transpose(out=z_nm[:, C512[j]], in_=h0_fd[:, C512[j]])

        u_ps = ps.tile([128, 2048], fp, tag="big")
        g1r = g1[:].rearrange("p (c nl h) -> p c h nl", c=2, nl=32, h=32)
        for cp in range(2):
            for hh in range(2):
                for c in range(2):
                    nc.tensor.matmul(
                        u_ps[:, cp * 1024 + 512 * hh : cp * 1024 + 512 * (hh + 1)],
                        atbd_s[:, 2 * c + cp, :],
                        g1r[:, c, 16 * hh : 16 * (hh + 1), :],
                        start=(c == 0),
                        stop=(c == 1),
                    )

        u_nm = st.tile([128, 2048], fp, tag="u_nm")
        ui = u_ps[:].rearrange("p (cp h nl) -> p cp h nl", cp=2, h=32, nl=32)
        uo = u_nm[:].rearrange("p (cp tl h) -> p cp h tl", cp=2, tl=32, h=32)
        for cp in range(2):
            for k in range(2):
                nc.vector.transpose(
                    out=uo[:, cp, 16 * k : 16 * (k + 1), :], in_=ui[:, cp, 16 * k : 16 * (k + 1), :]
                )

        zpre = ps.tile([128, 2048], fp, tag="big")
        for j in range(4):
            nc.tensor.matmul(zpre[:, C512[j]], pmat, z_nm[:, C512[j]], start=True, stop=False)
        for j in range(4):
            nc.tensor.matmul(zpre[:, C512[j]], qmat, u_nm[:, C512[j]], start=False, stop=True)

        zf = st.tile([128, 2048], fp, tag="zf")
        for j in range(4):
            nc.vector.transpose(out=zf[:, C512[j]], in_=zpre[:, C512[j]])

        pre1 = ps.tile([128, 2048], fp, tag="big")
        for j in range(4):
            nc.tensor.matmul(pre1[:, C512[j]], hi4_s[:, 0, :], zf[:, C512[j]], start=True, stop=True)
        h1_fd = st.tile([128, 2048], fp, tag="h1_fd")
        for j in range(4):
            nc.scalar.activation(h1_fd[:, C512[j]], pre1[:, C512[j]], AF.Tanh)

        # ====================== layer 2 (t-quarter only) ======================
        g2 = t_and_n_mix(h1_fd, "g", True)  # At-mix needs all t

        z2_nm = st.tile([128, 512], fp, tag="z2_nm")
        h1v = h1_fd[:].rearrange("p (t nl) -> p t nl", nl=32)
        nc.vector.transpose(out=z2_nm[:], in_=h1v[:, ds(toff, TQ), :])

        u2_ps = ps.tile([128, 1024], fp, tag="big")
        g2r = g2[:].rearrange("p (c nl h) -> p c h nl", c=2, nl=32, h=32)
        for hh in range(2):
            for c in range(2):
                nc.tensor.matmul(
                    u2_ps[:, 512 * hh : 512 * (hh + 1)],
                    atbq_s[:, c, :],
                    g2r[:, c, 16 * hh : 16 * (hh + 1), :],
                    start=(c == 0),
                    stop=(c == 1),
                )

        u2_nm = st.tile([128, 1024], fp, tag="u2_nm")
        u2i = u2_ps[:].rearrange("p (h nl) -> p h nl", h=32, nl=32)
        u2o = u2_nm[:].rearrange("p (i h) -> p h i", i=32, h=32)
        for k in range(2):
            nc.vector.transpose(
                out=u2o[:, 16 * k : 16 * (k + 1), :], in_=u2i[:, 16 * k : 16 * (k + 1), :]
            )

        zpre2 = ps.tile([128, 512], fp, tag="big")
        nc.tensor.matmul(zpre2[:], pmat, z2_nm[:], start=True, stop=False)
        nc.tensor.matmul(zpre2[:], qmat, u2_nm[:, 0:512], start=False, stop=True)

        z2f = st.tile([128, 512], fp, tag="z2f")
        nc.vector.transpose(out=z2f[:], in_=zpre2[:])

        pre2 = ps.tile([128, 512], fp, tag="big")
        nc.tensor.matmul(pre2[:], hi4_s[:, 1, :], z2f[:], start=True, stop=True)
        h2_fd = st.tile([128, 512], fp, tag="h2_fd")
        nc.scalar.activation(h2_fd[:], pre2[:], AF.Tanh)

        opre = ps.tile([128, 512], fp, tag="big")
        nc.tensor.matmul(opre[:], w2i4_s[:], h2_fd[:], start=True, stop=True)
        out_fd = st.tile([128, 512], fp, tag="out_fd")
        nc.scalar.activation(out_fd[:], opre[:], AF.Identity, bias=b2t_s[:, 0:1])

        out_nm = st.tile([128, 512], fp, tag="out_nm")
        nc.vector.transpose(out=out_nm[:], in_=out_fd[:])

        onv = out_nm[:].rearrange("p (i j2) -> p i j2", j2=32)
        ov = outb.ap().rearrange("(i n) j -> n i j", n=128)
        nc.sync.dma_start(ov, onv[:, :, 0:FOUT])

    nc.compile()
    return nc


def _host_weights(Adj_t, Adj_s, s, H, W1, b1, W2, b2):
    f4 = np.float32
    I4 = np.eye(4, dtype=f4)
    I128 = np.eye(128, dtype=f4)
    Heff = H.sum(axis=1).astype(f4)  # [2, 32, 32]

    P = (s[0] * I128 + s[1] * Adj_s).astype(f4)
    Q = (s[2] * I128 + s[3] * Adj_s).astype(f4)
    pq = np.stack([P, Q])

    w1i4 = np.kron(I4, W1.astype(f4))
    hi4 = np.stack([np.kron(I4, Heff[0]), np.kron(I4, Heff[1])])
    w2pad = np.zeros((32, 32), dtype=f4)
    w2pad[:, :FOUT] = W2
    w2i4 = np.kron(I4, w2pad)

    b1t = np.tile(b1.astype(f4), 4)[:, None]
    b2pad = np.zeros(32, dtype=f4)
    b2pad[:FOUT] = b2
    b2t = np.tile(b2pad, 4)[:, None]

    atbd = np.zeros((2, 2, 128, 128), dtype=f4)
    for c in range(2):
        for cp in range(2):
            atbd[c, cp] = np.kron(I4, Adj_t[32 * c : 32 * (c + 1), 32 * cp : 32 * (cp + 1)].astype(f4))

    atbq_all = np.zeros((NQ, 2, 128, 128), dtype=f4)
    for q in range(NQ):
        for c in range(2):
            blk = np.zeros((32, 32), dtype=f4)
            blk[:, :TQ] = Adj_t[32 * c : 32 * (c + 1), TQ * q : TQ * (q + 1)]
            atbq_all[q, c] = np.kron(I4, blk)

    return dict(w1i4=w1i4, b1t=b1t, atbd=atbd, pq=pq, hi4=hi4, w2i4=w2i4, b2t=b2t), atbq_all


def _in_maps(inputs):
    f4 = np.float32
    x = np.ascontiguousarray(np.asarray(inputs["x"], dtype=f4))
    shared, atbq_all = _host_weights(
        np.asarray(inputs["Adj_t"], dtype=f4),
        np.asarray(inputs["Adj_s"], dtype=f4),
        np.asarray(inputs["s"], dtype=f4),
        np.asarray(inputs["H"], dtype=f4),
        np.asarray(inputs["W1"], dtype=f4),
        np.asarray(inputs["b1"], dtype=f4),
        np.asarray(inputs["W2"], dtype=f4),
        np.asarray(inputs["b2"], dtype=f4),
    )
    maps = []
    for c in range(NCORES):
        b, q = c // NQ, c % NQ
        m = {"xb": np.ascontiguousarray(x[b]), "atbq": np.ascontiguousarray(atbq_all[q])}
        m.update({k: np.ascontiguousarray(v) for k, v in shared.items()})
        maps.append(m)
    return maps


def kernel(**inputs) -> np.ndarray:
    from concourse import bass_utils

    if "nc" not in _CACHE:
        _CACHE["nc"] = _build_nc()
    nc = _CACHE["nc"]

    maps = _in_maps(inputs)
    import os

    trace = bool(int(os.environ.get("GTCNN_TRACE", "0")))
    res = bass_utils.run_bass_kernel_spmd(
        nc,
        maps,
        core_ids=list(range(NCORES)),
        trace=trace,
        trace_cores=list(range(NCORES)) if trace else None,
        stitch_traces=False,
    )
    _CACHE["last_results"] = res

    out = np.empty((B, M, FOUT), dtype=np.float32)
    for c in range(NCORES):
        b, q = c // NQ, c % NQ
        out[b, 2048 * q : 2048 * (q + 1), :] = res.results[c]["outb"]
    return out



# revision 11
# speedup vs baseline: 1.0650x; 1.0650x over previous
"""Trainium2 Bass kernel for nn_GTCNN (product-graph GTCNN, 2 layers, K collapsed).

Math (per batch b, x: [M=8192, 32]):
  Adj = s0*I + s1*kron(I_t, As) + s2*kron(At, I_s) + s3*kron(At, As),  T=64, N=128
  h0 = x @ W1 + b1
  h_{l+1} = tanh((Adj @ h_l) @ Heff_l),   Heff_l = sum_k H[l, k]   (einsum collapses k)
  out = h2 @ W2 + b2

Device algorithm (Heff commutes with Adj, so Heff1 folds into W1 host-side):
  w  = x @ (W1 Heff1) + 1 (x) (b1 Heff1)          [FD matmul, fp32]
  z1 = tanh(P w + Q At-mix(w))                     [layer 1, all t, bf16]
  y  = P z1[:, q] + Q At[q,:]-mix(z1)              [layer 2, t-quarter]
  z2 = tanh(y @ Heff2);  out = z2 @ W2 + b2
  with P = s0*I + s1*As, Q = s2*I + s3*As folded on host.

Sharding: core c -> (b = c // 4, t-quarter q = c % 4). Layer 1 computed fully per
b (4x redundant, no collectives); layer 2 + output restricted to the 16-t quarter.

Layouts (n = 32*nh + nl, t = 32*c + tl):
  NM  [n, t*32 + h]                      node-on-partition (P / Q matmuls)
  FD  [32*nh + h,  t*32 + nl]            feature-on-partition (W1/Heff/W2 matmuls,
                                         block-diag kron(I4, W) stationaries)
  FDT [32*nh + tl, c*1024 + nl*32 + h]   t-on-partition (At matmuls, stationaries
                                         kron(I4, At 32x32 block), PSUM-accum over c)
All matmuls run bf16 (except the fp32 x @ W1H); PSUM evictions are scalar-engine
copies that cast fp32 -> bf16; layout moves are DVE 32x32 StreamTranspose ops on
bf16 SBUF tiles (StreamTranspose cannot cast, so evict-then-transpose).
"""

import numpy as np

T, NS, B, FIN, HID, FOUT = 64, 128, 2, 32, 32, 16
M = T * NS
NCORES, NQ = 8, 4
TQ = T // NQ  # 16 t's per quarter

_CACHE = {}

# bf16 weight pack layout (columns in wpk [128, 1280]):
#   slot i (128 cols each): atbd[2c+cp] i=0..3, atbq[c] i=4..5, P i=6, Q i=7,
#   hi4_2 i=8, w2i4 i=9
WPK_COLS = 1280


def _build_nc():
    from contextlib import ExitStack

    import concourse.mybir as mybir
    import concourse.tile as tile
    from concourse import bacc
    from concourse.bass import ds

    fp = mybir.dt.float32
    bf = mybir.dt.bfloat16
    AF = mybir.ActivationFunctionType

    nc = bacc.Bacc(
        "TRN2",
        target_bir_lowering=False,
        debug=False,
        enable_asserts=False,
        num_devices=NCORES,
    )

    xb = nc.dram_tensor("xb", [M, FIN], fp, kind="ExternalInput")
    w1h = nc.dram_tensor("w1h", [128, 128], fp, kind="ExternalInput")
    bias2 = nc.dram_tensor("bias2", [128, 2], fp, kind="ExternalInput")
    wpk = nc.dram_tensor("wpk", [128, WPK_COLS], bf, kind="ExternalInput")
    outb = nc.dram_tensor("outb", [TQ * NS, FOUT], fp, kind="ExternalOutput")

    C512 = [slice(512 * j, 512 * (j + 1)) for j in range(4)]

    with tile.TileContext(nc) as tc, ExitStack() as ctx:
        const = ctx.enter_context(tc.tile_pool(name="const", bufs=1))
        st = ctx.enter_context(tc.tile_pool(name="st", bufs=1))
        ps = ctx.enter_context(tc.tile_pool(name="ps", bufs=2, space="PSUM"))

        # Core id -> quarter offset register (used for the layer-2 column slice
        # inside a PE matmul AP, so load it on the PE engine).
        pid = nc.tensor.partition_id()
        toff = (pid % NQ) * TQ  # t-offset of this core's quarter

        # ---- constants to SBUF (W1H first: needed for the first matmul) ----
        w1h_s = const.tile([128, 128], fp, tag="w1h")
        nc.sync.dma_start(w1h_s[:], w1h.ap())
        bias_s = const.tile([128, 2], fp, tag="bias")
        nc.sync.dma_start(bias_s[:], bias2.ap())

        # ---- x load: NM [n, (t, f)], 4 t-chunks to pipeline vs transposes ----
        x_nm = st.tile([128, 2048], fp, tag="x_nm")
        xv = xb.ap().rearrange("(t n) f -> n t f", n=128)
        x_nm_v = x_nm[:].rearrange("p (t f) -> p t f", f=32)
        for j in range(4):
            nc.sync.dma_start(x_nm_v[:, 16 * j : 16 * (j + 1), :], xv[:, 16 * j : 16 * (j + 1), :])

        # ---- bf16 weight pack (needed only after the first matmuls) ----
        wpk_s = const.tile([128, WPK_COLS], bf, tag="wpk")
        nc.sync.dma_start(wpk_s[:], wpk.ap())
        atbd = wpk_s[:].rearrange("p (i c) -> p i c", c=128)
        pmat = atbd[:, 6, :]
        qmat = atbd[:, 7, :]
        hi4_2 = atbd[:, 8, :]
        w2i4 = atbd[:, 9, :]

        # ---- x -> FD (fp32 DVE transposes) ----
        x_fd = st.tile([128, 2048], fp, tag="x_fd")
        for j in range(4):
            nc.vector.transpose(out=x_fd[:, C512[j]], in_=x_nm[:, C512[j]])

        # ---- w = x @ W1H + b1H  (FD, fp32 matmul, ACT evicts to bf16) ----
        wpre = ps.tile([128, 2048], fp, tag="big")
        for j in range(4):
            nc.tensor.matmul(wpre[:, C512[j]], w1h_s[:], x_fd[:, C512[j]], start=True, stop=True)
        w_fd = st.tile([128, 2048], bf, tag="w_fd")
        for j in range(4):
            nc.scalar.activation(w_fd[:, C512[j]], wpre[:, C512[j]], AF.Identity, bias=bias_s[:, 0:1])

        # ---- w -> NM and w -> FDT (bf16 DVE passes) ----
        w_nm = st.tile([128, 2048], bf, tag="w_nm")
        for j in range(4):
            nc.vector.transpose(out=w_nm[:, C512[j]], in_=w_fd[:, C512[j]])

        g1 = st.tile([128, 2048], bf, tag="g1")
        gi = w_fd[:].rearrange("p (c tl nl) -> p c nl tl", c=2, tl=32, nl=32)
        go = g1[:].rearrange("p (c nl h) -> p c nl h", c=2, nl=32, h=32)
        for c in range(2):
            for k in range(2):
                nc.vector.transpose(
                    out=go[:, c, 16 * k : 16 * (k + 1), :], in_=gi[:, c, 16 * k : 16 * (k + 1), :]
                )

        # ---- u1 = At-mix(w) (FDT out, PSUM-accum over c) ----
        u_ps = ps.tile([128, 2048], fp, tag="big")
        g1r = g1[:].rearrange("p (c nl h) -> p c h nl", c=2, nl=32, h=32)
        for cp in range(2):
            for hh in range(2):
                for c in range(2):
                    nc.tensor.matmul(
                        u_ps[:, cp * 1024 + 512 * hh : cp * 1024 + 512 * (hh + 1)],
                        atbd[:, 2 * c + cp, :],
                        g1r[:, c, 16 * hh : 16 * (hh + 1), :],
                        start=(c == 0),
                        stop=(c == 1),
                    )

        # ---- evict u (ACT, cast bf16), then FDT -> NM on DVE ----
        u_fdt = st.tile([128, 2048], bf, tag="u_fdt")
        for j in range(4):
            nc.scalar.activation(u_fdt[:, C512[j]], u_ps[:, C512[j]], AF.Identity)

        u_nm = st.tile([128, 2048], bf, tag="u_nm")
        ui = u_fdt[:].rearrange("p (cp h nl) -> p cp h nl", cp=2, h=32, nl=32)
        uo = u_nm[:].rearrange("p (cp tl h) -> p cp h tl", cp=2, tl=32, h=32)
        for cp in range(2):
            for k in range(2):
                nc.vector.transpose(
                    out=uo[:, cp, 16 * k : 16 * (k + 1), :], in_=ui[:, cp, 16 * k : 16 * (k + 1), :]
                )

        # ---- zpre1 = P w + Q u (NM), tanh-evict -> z1_nm bf16 ----
        zpre = ps.tile([128, 2048], fp, tag="big")
        for j in range(4):
            nc.tensor.matmul(zpre[:, C512[j]], pmat, w_nm[:, C512[j]], start=True, stop=False)
        for j in range(4):
            nc.tensor.matmul(zpre[:, C512[j]], qmat, u_nm[:, C512[j]], start=False, stop=True)

        z1_nm = st.tile([128, 2048], bf, tag="z1_nm")
        for j in range(4):
            nc.scalar.activation(z1_nm[:, C512[j]], zpre[:, C512[j]], AF.Tanh)

        # ====================== layer 2 (t-quarter only) ======================
        # g2 = FDT(z1) (full t needed for the At contraction)
        g2 = st.tile([128, 2048], bf, tag="g2")
        zi = z1_nm[:].rearrange("p (c tl h) -> p c h tl", c=2, tl=32, h=32)
        zo = g2[:].rearrange("p (c nl h) -> p c h nl", c=2, nl=32, h=32)
        for c in range(2):
            for k in range(2):
                nc.vector.transpose(
                    out=zo[:, c, 16 * k : 16 * (k + 1), :], in_=zi[:, c, 16 * k : 16 * (k + 1), :]
                )

        # u2 = At[q rows]-mix(z1) (FDT out: tl' in 0..15 used)
        u2_ps = ps.tile([128, 1024], fp, tag="big")
        g2r = g2[:].rearrange("p (c nl h) -> p c h nl", c=2, nl=32, h=32)
        for hh in range(2):
            for c in range(2):
                nc.tensor.matmul(
                    u2_ps[:, 512 * hh : 512 * (hh + 1)],
                    atbd[:, 4 + c, :],
                    g2r[:, c, 16 * hh : 16 * (hh + 1), :],
                    start=(c == 0),
                    stop=(c == 1),
                )

        u2_f = st.tile([128, 1024], bf, tag="u2_f")
        for j in range(2):
            nc.scalar.activation(
                u2_f[:, 512 * j : 512 * (j + 1)], u2_ps[:, 512 * j : 512 * (j + 1)], AF.Identity
            )

        u2_nm = st.tile([128, 1024], bf, tag="u2_nm")
        u2i = u2_f[:].rearrange("p (h nl) -> p h nl", h=32, nl=32)
        u2o = u2_nm[:].rearrange("p (i h) -> p h i", i=32, h=32)
        for k in range(2):
            nc.vector.transpose(
                out=u2o[:, 16 * k : 16 * (k + 1), :], in_=u2i[:, 16 * k : 16 * (k + 1), :]
            )

        # zpre2 = P z1[:, q] + Q u2 (NM quarter), evict bf16
        z1v = z1_nm[:].rearrange("p (t h) -> p t h", h=32)
        zpre2 = ps.tile([128, 512], fp, tag="big")
        zq_in = z1v[:, ds(toff, TQ), :]
        nc.tensor.matmul(zpre2[:], pmat, zq_in, start=True, stop=False)
        nc.tensor.matmul(zpre2[:], qmat, u2_nm[:, 0:512], start=False, stop=True)

        zq_nm = st.tile([128, 512], bf, tag="zq_nm")
        nc.scalar.activation(zq_nm[:], zpre2[:], AF.Identity)

        # NM -> FD, then Heff2 matmul + tanh
        zq_fd = st.tile([128, 512], bf, tag="zq_fd")
        nc.vector.transpose(out=zq_fd[:], in_=zq_nm[:])

        pre2 = ps.tile([128, 512], fp, tag="big")
        nc.tensor.matmul(pre2[:], hi4_2, zq_fd[:], start=True, stop=True)
        h2_fd = st.tile([128, 512], bf, tag="h2_fd")
        nc.scalar.activation(h2_fd[:], pre2[:], AF.Tanh)

        # out = W2 h2 + b2 (FD), back to NM (fp32), DMA out
        opre = ps.tile([128, 512], fp, tag="big")
        nc.tensor.matmul(opre[:], w2i4, h2_fd[:], start=True, stop=True)
        out_fd = st.tile([128, 512], fp, tag="out_fd")
        nc.scalar.activation(out_fd[:], opre[:], AF.Identity, bias=bias_s[:, 1:2])

        out_nm = st.tile([128, 512], fp, tag="out_nm")
        nc.vector.transpose(out=out_nm[:], in_=out_fd[:])

        onv = out_nm[:].rearrange("p (i j2) -> p i j2", j2=32)
        ov = outb.ap().rearrange("(i n) j -> n i j", n=128)
        nc.sync.dma_start(ov, onv[:, :, 0:FOUT])

    nc.compile()
    return nc


def _host_weights(Adj_t, Adj_s, s, H, W1, b1, W2, b2):
    import ml_dtypes

    f4 = np.float32
    bf = ml_dtypes.bfloat16
    I4 = np.eye(4, dtype=f4)
    I128 = np.eye(128, dtype=f4)
    Heff = H.sum(axis=1).astype(f4)  # [2, 32, 32]

    P = (s[0] * I128 + s[1] * Adj_s).astype(f4)
    Q = (s[2] * I128 + s[3] * Adj_s).astype(f4)

    W1H = (W1 @ Heff[0]).astype(f4)
    b1H = (b1 @ Heff[0]).astype(f4)
    w1h = np.kron(I4, W1H)

    hi4_2 = np.kron(I4, Heff[1])
    w2pad = np.zeros((32, 32), dtype=f4)
    w2pad[:, :FOUT] = W2
    w2i4 = np.kron(I4, w2pad)

    bias2 = np.zeros((128, 2), dtype=f4)
    bias2[:, 0] = np.tile(b1H, 4)
    b2pad = np.zeros(32, dtype=f4)
    b2pad[:FOUT] = b2
    bias2[:, 1] = np.tile(b2pad, 4)

    # Per-quarter bf16 packs; 128-col slot i of wpk: atbd[2c+cp] in 0..3,
    # atbq[c] in 4..5, P=6, Q=7, hi4_2=8, w2i4=9.
    wpk = np.zeros((NQ, 128, WPK_COLS), dtype=bf)
    for c in range(2):
        for cp in range(2):
            blk = np.kron(I4, Adj_t[32 * c : 32 * (c + 1), 32 * cp : 32 * (cp + 1)].astype(f4))
            wpk[:, :, 128 * (2 * c + cp) : 128 * (2 * c + cp + 1)] = blk.astype(bf)
    for q in range(NQ):
        for c in range(2):
            blk = np.zeros((32, 32), dtype=f4)
            blk[:, :TQ] = Adj_t[32 * c : 32 * (c + 1), TQ * q : TQ * (q + 1)]
            wpk[q, :, 128 * (4 + c) : 128 * (5 + c)] = np.kron(I4, blk).astype(bf)
    wpk[:, :, 128 * 6 : 128 * 7] = P.astype(bf)
    wpk[:, :, 128 * 7 : 128 * 8] = Q.astype(bf)
    wpk[:, :, 128 * 8 : 128 * 9] = hi4_2.astype(bf)
    wpk[:, :, 128 * 9 : 128 * 10] = w2i4.astype(bf)

    return w1h, bias2, wpk


def _in_maps(inputs):
    f4 = np.float32
    x = np.ascontiguousarray(np.asarray(inputs["x"], dtype=f4))
    w1h, bias2, wpk = _host_weights(
        np.asarray(inputs["Adj_t"], dtype=f4),
        np.asarray(inputs["Adj_s"], dtype=f4),
        np.asarray(inputs["s"], dtype=f4),
        np.asarray(inputs["H"], dtype=f4),
        np.asarray(inputs["W1"], dtype=f4),
        np.asarray(inputs["b1"], dtype=f4),
        np.asarray(inputs["W2"], dtype=f4),
        np.asarray(inputs["b2"], dtype=f4),
    )
    maps = []
    for c in range(NCORES):
        b, q = c // NQ, c % NQ
        maps.append(
            {
                "xb": np.ascontiguousarray(x[b]),
                "w1h": w1h,
                "bias2": bias2,
                "wpk": np.ascontiguousarray(wpk[q]),
            }
        )
    return maps


def kernel(**inputs) -> np.ndarray:
    from concourse import bass_utils

    if "nc" not in _CACHE:
        _CACHE["nc"] = _build_nc()
    nc = _CACHE["nc"]

    maps = _in_maps(inputs)
    import os

    trace = bool(int(os.environ.get("GTCNN_TRACE", "0")))
    res = bass_utils.run_bass_kernel_spmd(
        nc,
        maps,
        core_ids=list(range(NCORES)),
        trace=trace,
        trace_cores=list(range(NCORES)) if trace else None,
        stitch_traces=False,
    )
    _CACHE["last_results"] = res

    out = np.empty((B, M, FOUT), dtype=np.float32)
    for c in range(NCORES):
        b, q = c // NQ, c % NQ
        out[b, 2048 * q : 2048 * (q + 1), :] = res.results[c]["outb"]
    return out


# revision 17
# speedup vs baseline: 1.2023x; 1.1289x over previous
"""Trainium2 Bass kernel for nn_GTCNN (product-graph GTCNN, 2 layers, K collapsed).

Math (per batch b, x: [M=8192, 32]):
  Adj = s0*I + s1*kron(I_t, As) + s2*kron(At, I_s) + s3*kron(At, As),  T=64, N=128
  h0 = x @ W1 + b1
  h_{l+1} = tanh((Adj @ h_l) @ Heff_l),   Heff_l = sum_k H[l, k]   (einsum collapses k)
  out = h2 @ W2 + b2

Device algorithm (Heff commutes with Adj, so Heff1 folds into W1 host-side):
  w  = x @ (W1 Heff1) + 1 (x) (b1 Heff1)          [FD matmul, fp32]
  z1 = tanh(P w + Q At-mix(w))                     [layer 1, all t, bf16]
  y  = P z1[:, q] + Q At[q,:]-mix(z1)              [layer 2, t-quarter]
  z2 = tanh(y @ Heff2);  out = z2 @ W2 + b2
  with P = s0*I + s1*As, Q = s2*I + s3*As folded on host.

Sharding: core c -> (b = c // 4, t-quarter q = c % 4). Layer 1 computed fully per
b (4x redundant, no collectives); layer 2 + output restricted to the 16-t quarter.

Layouts (n = 32*nh + nl, t = 32*c + tl):
  NM  [n, t*32 + h]                      node-on-partition (P / Q matmuls)
  FD  [32*nh + h,  t*32 + nl]            feature-on-partition (W1/Heff/W2 matmuls,
                                         block-diag kron(I4, W) stationaries)
  FDT [32*nh + tl, c*1024 + nl*32 + h]   t-on-partition (At matmuls, stationaries
                                         kron(I4, At 32x32 block), PSUM-accum over c)
All matmuls run bf16 (except the fp32 x @ W1H); PSUM evictions are scalar-engine
copies that cast fp32 -> bf16; layout moves are DVE 32x32 StreamTranspose ops on
bf16 SBUF tiles (StreamTranspose cannot cast, so evict-then-transpose).
"""

import numpy as np

T, NS, B, FIN, HID, FOUT = 64, 128, 2, 32, 32, 16
M = T * NS
NCORES, NQ = 8, 4
TQ = T // NQ  # 16 t's per quarter

_CACHE = {}

# bf16 weight pack layout (columns in wpk [128, 1280]):
#   slot i (128 cols each): atbd[2c+cp] i=0..3, atbq[c] i=4..5, P i=6, Q i=7,
#   hi4_2 i=8, w2i4 i=9
WPK_COLS = 1280


def _build_nc():
    from contextlib import ExitStack

    import concourse.mybir as mybir
    import concourse.tile as tile
    from concourse import bacc
    from concourse.bass import ds

    fp = mybir.dt.float32
    bf = mybir.dt.bfloat16
    AF = mybir.ActivationFunctionType

    nc = bacc.Bacc(
        "TRN2",
        target_bir_lowering=False,
        debug=False,
        enable_asserts=False,
        num_devices=NCORES,
    )

    xb = nc.dram_tensor("xb", [M, FIN], fp, kind="ExternalInput")
    w1h = nc.dram_tensor("w1h", [128, 128], fp, kind="ExternalInput")
    bias2 = nc.dram_tensor("bias2", [128, 2], fp, kind="ExternalInput")
    wpk = nc.dram_tensor("wpk", [128, WPK_COLS], bf, kind="ExternalInput")
    outb = nc.dram_tensor("outb", [TQ * NS, FOUT], fp, kind="ExternalOutput")

    C512 = [slice(512 * j, 512 * (j + 1)) for j in range(4)]

    with tile.TileContext(nc) as tc, ExitStack() as ctx:
        const = ctx.enter_context(tc.tile_pool(name="const", bufs=1))
        st = ctx.enter_context(tc.tile_pool(name="st", bufs=1))
        ps = ctx.enter_context(tc.tile_pool(name="ps", bufs=2, space="PSUM"))

        # Core id -> quarter offset register (used for the layer-2 column slice
        # inside a PE matmul AP, so load it on the PE engine).
        pid = nc.tensor.partition_id()
        toff = (pid % NQ) * TQ  # t-offset of this core's quarter

        # ---- constants to SBUF (W1H first: needed for the first matmul) ----
        w1h_s = const.tile([128, 128], fp, tag="w1h")
        nc.sync.dma_start(w1h_s[:], w1h.ap())
        bias_s = const.tile([128, 2], fp, tag="bias")
        nc.sync.dma_start(bias_s[:], bias2.ap())

        # ---- x load: NM [n, (t, f)], 4 t-chunks to pipeline vs transposes ----
        x_nm = st.tile([128, 2048], fp, tag="x_nm")
        xv = xb.ap().rearrange("(t n) f -> n t f", n=128)
        x_nm_v = x_nm[:].rearrange("p (t f) -> p t f", f=32)
        for j in range(4):
            nc.sync.dma_start(x_nm_v[:, 16 * j : 16 * (j + 1), :], xv[:, 16 * j : 16 * (j + 1), :])

        # ---- bf16 weight pack (needed only after the first matmuls) ----
        wpk_s = const.tile([128, WPK_COLS], bf, tag="wpk")
        nc.sync.dma_start(wpk_s[:], wpk.ap())
        atbd = wpk_s[:].rearrange("p (i c) -> p i c", c=128)
        pmat = atbd[:, 6, :]
        qmat = atbd[:, 7, :]
        hi4_2 = atbd[:, 8, :]
        w2i4 = atbd[:, 9, :]

        # ---- PE warm-up: dummy matmuls on weight data while x loads, to
        # release the HAM clock-gate (PE runs ~2x slower until ~4us of
        # sustained activity). Results land in a scratch PSUM bank, never read.
        warm_ps = ps.tile([128, 512], fp, tag="big")
        for _ in range(8):
            nc.tensor.matmul(warm_ps[:], wpk_s[:, 0:128], wpk_s[:, 0:512], start=True, stop=True)

        # ---- x -> FD (fp32 DVE transposes) ----
        x_fd = st.tile([128, 2048], fp, tag="x_fd")
        for j in range(4):
            nc.vector.transpose(out=x_fd[:, C512[j]], in_=x_nm[:, C512[j]])

        # ---- w = x @ W1H + b1H  (FD, fp32 matmul, ACT evicts to bf16) ----
        wpre = ps.tile([128, 2048], fp, tag="big")
        for j in range(4):
            nc.tensor.matmul(wpre[:, C512[j]], w1h_s[:], x_fd[:, C512[j]], start=True, stop=True)
        w_fd = st.tile([128, 2048], bf, tag="w_fd")
        for j in range(4):
            nc.scalar.activation(w_fd[:, C512[j]], wpre[:, C512[j]], AF.Identity, bias=bias_s[:, 0:1])

        # ---- w -> NM and w -> FDT (bf16 DVE passes) ----
        w_nm = st.tile([128, 2048], bf, tag="w_nm")
        for j in range(4):
            nc.vector.transpose(out=w_nm[:, C512[j]], in_=w_fd[:, C512[j]])

        g1 = st.tile([128, 2048], bf, tag="g1")
        gi = w_fd[:].rearrange("p (c tl nl) -> p c nl tl", c=2, tl=32, nl=32)
        go = g1[:].rearrange("p (c nl h) -> p c nl h", c=2, nl=32, h=32)
        for c in range(2):
            for k in range(2):
                nc.vector.transpose(
                    out=go[:, c, 16 * k : 16 * (k + 1), :], in_=gi[:, c, 16 * k : 16 * (k + 1), :]
                )

        # ---- u1 = At-mix(w) (FDT out, PSUM-accum over c) ----
        u_ps = ps.tile([128, 2048], fp, tag="big")
        g1r = g1[:].rearrange("p (c nl h) -> p c h nl", c=2, nl=32, h=32)
        for cp in range(2):
            for hh in range(2):
                for c in range(2):
                    nc.tensor.matmul(
                        u_ps[:, cp * 1024 + 512 * hh : cp * 1024 + 512 * (hh + 1)],
                        atbd[:, 2 * c + cp, :],
                        g1r[:, c, 16 * hh : 16 * (hh + 1), :],
                        start=(c == 0),
                        stop=(c == 1),
                    )

        # ---- evict u (ACT, cast bf16), then FDT -> NM on DVE ----
        u_fdt = st.tile([128, 2048], bf, tag="u_fdt")
        for j in range(4):
            nc.scalar.activation(u_fdt[:, C512[j]], u_ps[:, C512[j]], AF.Identity)

        # u_nm stored t-inner (col = 64h + 32cp + tl) so the transpose writes
        # stride-1 (strided DVE *writes* cost ~3.6x; strided reads are cheap).
        u_nm = st.tile([128, 2048], bf, tag="u_nm")
        ui = u_fdt[:].rearrange("p (cp h nl) -> p cp h nl", cp=2, h=32, nl=32)
        uo = u_nm[:].rearrange("p (h cp tl) -> p cp h tl", h=32, cp=2, tl=32)
        for cp in range(2):
            for k in range(2):
                nc.vector.transpose(
                    out=uo[:, cp, 16 * k : 16 * (k + 1), :], in_=ui[:, cp, 16 * k : 16 * (k + 1), :]
                )

        # ---- zpre1 = P w + Q u (NM), tanh-evict -> z1_nm bf16 ----
        u_mv = u_nm[:].rearrange("p (h cp tl) -> p cp tl h", h=32, cp=2, tl=32)
        zpre = ps.tile([128, 2048], fp, tag="big")
        for j in range(4):
            nc.tensor.matmul(zpre[:, C512[j]], pmat, w_nm[:, C512[j]], start=True, stop=False)
        for j in range(4):
            nc.tensor.matmul(
                zpre[:, C512[j]],
                qmat,
                u_mv[:, j // 2, 16 * (j % 2) : 16 * (j % 2) + 16, :],
                start=False,
                stop=True,
            )

        z1_nm = st.tile([128, 2048], bf, tag="z1_nm")
        for j in range(4):
            nc.scalar.activation(z1_nm[:, C512[j]], zpre[:, C512[j]], AF.Tanh)

        # ====================== layer 2 (t-quarter only) ======================
        # g2 = FDT(z1) (full t needed for the At contraction)
        # g2 stored nl-inner (col = 1024c + 32h + nl) for stride-1 writes.
        g2 = st.tile([128, 2048], bf, tag="g2")
        zi = z1_nm[:].rearrange("p (c tl h) -> p c h tl", c=2, tl=32, h=32)
        zo = g2[:].rearrange("p (c h nl) -> p c h nl", c=2, h=32, nl=32)
        for c in range(2):
            for k in range(2):
                nc.vector.transpose(
                    out=zo[:, c, 16 * k : 16 * (k + 1), :], in_=zi[:, c, 16 * k : 16 * (k + 1), :]
                )

        # u2 = At[q rows]-mix(z1) (FDT out: tl' in 0..15 used)
        u2_ps = ps.tile([128, 1024], fp, tag="big")
        g2r = g2[:].rearrange("p (c h nl) -> p c h nl", c=2, h=32, nl=32)
        for hh in range(2):
            for c in range(2):
                nc.tensor.matmul(
                    u2_ps[:, 512 * hh : 512 * (hh + 1)],
                    atbd[:, 4 + c, :],
                    g2r[:, c, 16 * hh : 16 * (hh + 1), :],
                    start=(c == 0),
                    stop=(c == 1),
                )

        u2_f = st.tile([128, 1024], bf, tag="u2_f")
        for j in range(2):
            nc.scalar.activation(
                u2_f[:, 512 * j : 512 * (j + 1)], u2_ps[:, 512 * j : 512 * (j + 1)], AF.Identity
            )

        # u2_nm stored i-inner (col = 32h + i) for stride-1 writes.
        u2_nm = st.tile([128, 1024], bf, tag="u2_nm")
        u2i = u2_f[:].rearrange("p (h nl) -> p h nl", h=32, nl=32)
        u2o = u2_nm[:].rearrange("p (h i) -> p h i", h=32, i=32)
        for k in range(2):
            nc.vector.transpose(
                out=u2o[:, 16 * k : 16 * (k + 1), :], in_=u2i[:, 16 * k : 16 * (k + 1), :]
            )

        # zpre2 = P z1[:, q] + Q u2 (NM quarter), evict bf16
        z1v = z1_nm[:].rearrange("p (t h) -> p t h", h=32)
        zpre2 = ps.tile([128, 512], fp, tag="big")
        zq_in = z1v[:, ds(toff, TQ), :]
        u2_mv = u2_nm[:].rearrange("p (h i) -> p i h", h=32, i=32)
        nc.tensor.matmul(zpre2[:], pmat, zq_in, start=True, stop=False)
        nc.tensor.matmul(zpre2[:], qmat, u2_mv[:, 0:16, :], start=False, stop=True)

        zq_nm = st.tile([128, 512], bf, tag="zq_nm")
        nc.scalar.activation(zq_nm[:], zpre2[:], AF.Identity)

        # NM -> FD, then Heff2 matmul + tanh
        zq_fd = st.tile([128, 512], bf, tag="zq_fd")
        nc.vector.transpose(out=zq_fd[:], in_=zq_nm[:])

        pre2 = ps.tile([128, 512], fp, tag="big")
        nc.tensor.matmul(pre2[:], hi4_2, zq_fd[:], start=True, stop=True)
        h2_fd = st.tile([128, 512], bf, tag="h2_fd")
        nc.scalar.activation(h2_fd[:], pre2[:], AF.Tanh)

        # out = W2 h2 + b2 (FD), back to NM (fp32), DMA out
        opre = ps.tile([128, 512], fp, tag="big")
        nc.tensor.matmul(opre[:], w2i4, h2_fd[:], start=True, stop=True)
        out_fd = st.tile([128, 512], fp, tag="out_fd")
        nc.scalar.activation(out_fd[:], opre[:], AF.Identity, bias=bias_s[:, 1:2])

        out_nm = st.tile([128, 512], fp, tag="out_nm")
        nc.vector.transpose(out=out_nm[:], in_=out_fd[:])

        onv = out_nm[:].rearrange("p (i j2) -> p i j2", j2=32)
        ov = outb.ap().rearrange("(i n) j -> n i j", n=128)
        nc.sync.dma_start(ov, onv[:, :, 0:FOUT])

    nc.compile()
    return nc


def _host_weights(Adj_t, Adj_s, s, H, W1, b1, W2, b2):
    import ml_dtypes

    f4 = np.float32
    bf = ml_dtypes.bfloat16
    I4 = np.eye(4, dtype=f4)
    I128 = np.eye(128, dtype=f4)
    Heff = H.sum(axis=1).astype(f4)  # [2, 32, 32]

    P = (s[0] * I128 + s[1] * Adj_s).astype(f4)
    Q = (s[2] * I128 + s[3] * Adj_s).astype(f4)

    W1H = (W1 @ Heff[0]).astype(f4)
    b1H = (b1 @ Heff[0]).astype(f4)
    w1h = np.kron(I4, W1H)

    hi4_2 = np.kron(I4, Heff[1])
    w2pad = np.zeros((32, 32), dtype=f4)
    w2pad[:, :FOUT] = W2
    w2i4 = np.kron(I4, w2pad)

    bias2 = np.zeros((128, 2), dtype=f4)
    bias2[:, 0] = np.tile(b1H, 4)
    b2pad = np.zeros(32, dtype=f4)
    b2pad[:FOUT] = b2
    bias2[:, 1] = np.tile(b2pad, 4)

    # Per-quarter bf16 packs; 128-col slot i of wpk: atbd[2c+cp] in 0..3,
    # atbq[c] in 4..5, P=6, Q=7, hi4_2=8, w2i4=9.
    wpk = np.zeros((NQ, 128, WPK_COLS), dtype=bf)
    for c in range(2):
        for cp in range(2):
            blk = np.kron(I4, Adj_t[32 * c : 32 * (c + 1), 32 * cp : 32 * (cp + 1)].astype(f4))
            wpk[:, :, 128 * (2 * c + cp) : 128 * (2 * c + cp + 1)] = blk.astype(bf)
    for q in range(NQ):
        for c in range(2):
            blk = np.zeros((32, 32), dtype=f4)
            blk[:, :TQ] = Adj_t[32 * c : 32 * (c + 1), TQ * q : TQ * (q + 1)]
            wpk[q, :, 128 * (4 + c) : 128 * (5 + c)] = np.kron(I4, blk).astype(bf)
    wpk[:, :, 128 * 6 : 128 * 7] = P.astype(bf)
    wpk[:, :, 128 * 7 : 128 * 8] = Q.astype(bf)
    wpk[:, :, 128 * 8 : 128 * 9] = hi4_2.astype(bf)
    wpk[:, :, 128 * 9 : 128 * 10] = w2i4.astype(bf)

    return w1h, bias2, wpk


def _in_maps(inputs):
    f4 = np.float32
    x = np.ascontiguousarray(np.asarray(inputs["x"], dtype=f4))
    w1h, bias2, wpk = _host_weights(
        np.asarray(inputs["Adj_t"], dtype=f4),
        np.asarray(inputs["Adj_s"], dtype=f4),
        np.asarray(inputs["s"], dtype=f4),
        np.asarray(inputs["H"], dtype=f4),
        np.asarray(inputs["W1"], dtype=f4),
        np.asarray(inputs["b1"], dtype=f4),
        np.asarray(inputs["W2"], dtype=f4),
        np.asarray(inputs["b2"], dtype=f4),
    )
    maps = []
    for c in range(NCORES):
        b, q = c // NQ, c % NQ
        maps.append(
            {
                "xb": np.ascontiguousarray(x[b]),
                "w1h": w1h,
                "bias2": bias2,
                "wpk": np.ascontiguousarray(wpk[q]),
            }
        )
    return maps


def kernel(**inputs) -> np.ndarray:
    from concourse import bass_utils

    if "nc" not in _CACHE:
        _CACHE["nc"] = _build_nc()
    nc = _CACHE["nc"]

    maps = _in_maps(inputs)
    import os

    trace = bool(int(os.environ.get("GTCNN_TRACE", "0")))
    res = bass_utils.run_bass_kernel_spmd(
        nc,
        maps,
        core_ids=list(range(NCORES)),
        trace=trace,
        trace_cores=list(range(NCORES)) if trace else None,
        stitch_traces=False,
    )
    _CACHE["last_results"] = res

    out = np.empty((B, M, FOUT), dtype=np.float32)
    for c in range(NCORES):
        b, q = c // NQ, c % NQ
        out[b, 2048 * q : 2048 * (q + 1), :] = res.results[c]["outb"]
    return out


# revision 19
# speedup vs baseline: 1.2444x; 1.0351x over previous
"""Trainium2 Bass kernel for nn_GTCNN (product-graph GTCNN, 2 layers, K collapsed).

Math (per batch b, x: [M=8192, 32]):
  Adj = s0*I + s1*kron(I_t, As) + s2*kron(At, I_s) + s3*kron(At, As),  T=64, N=128
  h0 = x @ W1 + b1
  h_{l+1} = tanh((Adj @ h_l) @ Heff_l),   Heff_l = sum_k H[l, k]
  out = h2 @ W2 + b2

Device algorithm (Heff commutes with Adj, so Heff1 folds into W1 host-side):
  w  = x @ (W1 Heff1) + 1 (x) (b1 Heff1)          [FD matmul]
  z1 = tanh(P w + Q At-mix(w))                     [layer 1, all t]
  y  = P z1[:, q] + Q At[q,:]-mix(z1)              [layer 2, t-quarter]
  z2 = tanh(y @ Heff2);  out = z2 @ W2 + b2
  with P = s0*I + s1*As, Q = s2*I + s3*As folded on host.

Sharding: core c -> (b = c // 4, t-quarter q = c % 4). Layer 1 computed fully per
b (4x redundant; collectives have a ~10us floor, far above the redundant work).

Layouts (n = 32*nh + nl, t = 32*c + tl):
  NM  [n, t*32 + h]                      node-on-partition (P / Q matmuls)
  FD  [32*nh + h,  t*32 + nl]            feature-on-partition (W/Heff stationaries
                                         are block-diag kron(I4, W))
  FDT [32*nh + tl, ...]                  t-on-partition (At matmuls, stationaries
                                         kron(I4, At 32x32 block), PSUM-accum c)
All matmuls bf16 (PSUM fp32); PSUM evictions are scalar-engine copies casting to
bf16; layout moves are DVE 32x32 StreamTranspose ops on bf16 SBUF tiles. Every
transpose WRITES with stride-1 within-block (strided DVE writes cost ~3.6x);
consuming matmuls absorb the resulting layout via strided moving-AP views.
Tiles that land transposed-conventions: u_nm col = 64h+32cp+tl, g2 col =
1024c+32h+nl, u2_nm col = 32h+i.
"""

import numpy as np

T, NS, B, FIN, HID, FOUT = 64, 128, 2, 32, 32, 16
M = T * NS
NCORES, NQ = 8, 4
TQ = T // NQ  # 16 t's per quarter

_CACHE = {}

# bf16 weight pack [128, 1408]; 128-col slot i: atbd[2c+cp] i=0..3, atbq[c]
# i=4..5, P i=6, Q i=7, hi4_2 i=8, w2i4 i=9, w1hi4 i=10.
WPK_COLS = 1408


def _build_nc():
    from contextlib import ExitStack

    import concourse.mybir as mybir
    import concourse.tile as tile
    from concourse import bacc
    from concourse.bass import ds

    fp = mybir.dt.float32
    bf = mybir.dt.bfloat16
    AF = mybir.ActivationFunctionType

    nc = bacc.Bacc(
        "TRN2",
        target_bir_lowering=False,
        debug=False,
        enable_asserts=False,
        num_devices=NCORES,
    )

    xb = nc.dram_tensor("xb", [M, FIN], fp, kind="ExternalInput")
    bias2 = nc.dram_tensor("bias2", [128, 2], fp, kind="ExternalInput")
    wpk = nc.dram_tensor("wpk", [128, WPK_COLS], bf, kind="ExternalInput")
    outb = nc.dram_tensor("outb", [TQ * NS, FOUT], fp, kind="ExternalOutput")

    C512 = [slice(512 * j, 512 * (j + 1)) for j in range(4)]
    C1024 = [slice(1024 * j, 1024 * (j + 1)) for j in range(2)]

    with tile.TileContext(nc) as tc, ExitStack() as ctx:
        const = ctx.enter_context(tc.tile_pool(name="const", bufs=1))
        st = ctx.enter_context(tc.tile_pool(name="st", bufs=1))
        ps = ctx.enter_context(tc.tile_pool(name="ps", bufs=2, space="PSUM"))

        # Quarter offset register, loaded on PE (used inside a matmul AP).
        pid = nc.tensor.partition_id()
        toff = (pid % NQ) * TQ

        # ---- PE warm-up on an uninitialized tile: no input deps, so these
        # run at t~0 while DMAs stream, releasing the HAM clock-gate (PE is
        # ~2x slower until ~4us of sustained activity). Output never read.
        junk = const.tile([128, 512], bf, tag="junk")
        nc.gpsimd.memset(junk[:], 0)
        warm_ps = ps.tile([128, 512], fp, tag="big")
        for _ in range(14):
            nc.tensor.matmul(warm_ps[:], junk[:, 0:128], junk[:], start=True, stop=True)

        # ---- x load first (the critical path): NM [n, (t, f)], 4 t-chunks
        # split across the two HWDGE rings (sync + scalar queues).
        x_nm = st.tile([128, 2048], fp, tag="x_nm")
        xv = xb.ap().rearrange("(t n) f -> n t f", n=128)
        x_nm_v = x_nm[:].rearrange("p (t f) -> p t f", f=32)
        for j in range(4):
            eng = nc.sync if j % 2 == 0 else nc.scalar
            eng.dma_start(x_nm_v[:, 16 * j : 16 * (j + 1), :], xv[:, 16 * j : 16 * (j + 1), :])

        bias_s = const.tile([128, 2], fp, tag="bias")
        nc.sync.dma_start(bias_s[:], bias2.ap())
        wpk_s = const.tile([128, WPK_COLS], bf, tag="wpk")
        nc.scalar.dma_start(wpk_s[:], wpk.ap())
        wslot = wpk_s[:].rearrange("p (i c) -> p i c", c=128)
        pmat = wslot[:, 6, :]
        qmat = wslot[:, 7, :]
        hi4_2 = wslot[:, 8, :]
        w2i4 = wslot[:, 9, :]
        w1hi4 = wslot[:, 10, :]

        # ---- per chunk: cast x to bf16 (ACT), NM -> FD (DVE) ----
        x_nmb = st.tile([128, 2048], bf, tag="x_nmb")
        for j in range(4):
            nc.scalar.activation(x_nmb[:, C512[j]], x_nm[:, C512[j]], AF.Identity)
        x_fd = st.tile([128, 2048], bf, tag="x_fd")
        for j in range(4):
            nc.vector.transpose(out=x_fd[:, C512[j]], in_=x_nmb[:, C512[j]])

        # ---- w = x @ W1H + b1H  (FD), ACT bias-evict to bf16 ----
        wpre = ps.tile([128, 2048], fp, tag="big")
        for j in range(4):
            nc.tensor.matmul(wpre[:, C512[j]], w1hi4, x_fd[:, C512[j]], start=True, stop=True)
        w_fd = st.tile([128, 2048], bf, tag="w_fd")
        for j in range(4):
            nc.scalar.activation(w_fd[:, C512[j]], wpre[:, C512[j]], AF.Identity, bias=bias_s[:, 0:1])

        # ---- w -> NM (std) and w -> FDT (std g convention: 1024c+32nl+h) ----
        w_nm = st.tile([128, 2048], bf, tag="w_nm")
        for j in range(2):
            nc.vector.transpose(out=w_nm[:, C1024[j]], in_=w_fd[:, C1024[j]])

        g1 = st.tile([128, 2048], bf, tag="g1")
        gi = w_fd[:].rearrange("p (c tl nl) -> p c nl tl", c=2, tl=32, nl=32)
        go = g1[:].rearrange("p (c nl h) -> p c nl h", c=2, nl=32, h=32)
        for c in range(2):
            nc.vector.transpose(out=go[:, c], in_=gi[:, c])

        # ---- u1 = At-mix(w) (FDT out, PSUM-accum over c) ----
        u_ps = ps.tile([128, 2048], fp, tag="big")
        g1r = g1[:].rearrange("p (c nl h) -> p c h nl", c=2, nl=32, h=32)
        for cp in range(2):
            for hh in range(2):
                for c in range(2):
                    nc.tensor.matmul(
                        u_ps[:, cp * 1024 + 512 * hh : cp * 1024 + 512 * (hh + 1)],
                        wslot[:, 2 * c + cp, :],
                        g1r[:, c, 16 * hh : 16 * (hh + 1), :],
                        start=(c == 0),
                        stop=(c == 1),
                    )

        # ---- ACT-evict u (cast bf16), FDT -> NM on DVE (stride-1 writes:
        # u_nm stored t-inner, col = 64h + 32cp + tl) ----
        u_fdt = st.tile([128, 2048], bf, tag="u_fdt")
        for j in range(2):
            nc.scalar.activation(u_fdt[:, C1024[j]], u_ps[:, C1024[j]], AF.Identity)

        u_nm = st.tile([128, 2048], bf, tag="u_nm")
        ui = u_fdt[:].rearrange("p (cp h nl) -> p cp h nl", cp=2, h=32, nl=32)
        uo = u_nm[:].rearrange("p (h cp tl) -> p cp h tl", h=32, cp=2, tl=32)
        for cp in range(2):
            nc.vector.transpose(out=uo[:, cp], in_=ui[:, cp])

        # ---- zpre1 = P w + Q u (NM), tanh-evict -> z1_nm bf16 ----
        u_mv = u_nm[:].rearrange("p (h cp tl) -> p cp tl h", h=32, cp=2, tl=32)
        zpre = ps.tile([128, 2048], fp, tag="big")
        for j in range(4):
            nc.tensor.matmul(zpre[:, C512[j]], pmat, w_nm[:, C512[j]], start=True, stop=False)
            nc.tensor.matmul(
                zpre[:, C512[j]],
                qmat,
                u_mv[:, j // 2, 16 * (j % 2) : 16 * (j % 2) + 16, :],
                start=False,
                stop=True,
            )

        z1_nm = st.tile([128, 2048], bf, tag="z1_nm")
        for j in range(2):
            nc.scalar.activation(z1_nm[:, C1024[j]], zpre[:, C1024[j]], AF.Tanh)

        # ====================== layer 2 (t-quarter only) ======================
        # g2 = FDT'(z1), stored nl-inner: col = 1024c + 32h + nl.
        g2 = st.tile([128, 2048], bf, tag="g2")
        zi = z1_nm[:].rearrange("p (c tl h) -> p c h tl", c=2, tl=32, h=32)
        zo = g2[:].rearrange("p (c h nl) -> p c h nl", c=2, h=32, nl=32)
        for c in range(2):
            nc.vector.transpose(out=zo[:, c], in_=zi[:, c])

        # u2 = At[q rows]-mix(z1): out partitions (nh, tl' in 0..15)
        u2_ps = ps.tile([128, 1024], fp, tag="big")
        g2r = g2[:].rearrange("p (c h nl) -> p c h nl", c=2, h=32, nl=32)
        for hh in range(2):
            for c in range(2):
                nc.tensor.matmul(
                    u2_ps[:, 512 * hh : 512 * (hh + 1)],
                    wslot[:, 4 + c, :],
                    g2r[:, c, 16 * hh : 16 * (hh + 1), :],
                    start=(c == 0),
                    stop=(c == 1),
                )

        u2_f = st.tile([128, 1024], bf, tag="u2_f")
        nc.scalar.activation(u2_f[:], u2_ps[:], AF.Identity)

        # u2_nm stored i-inner (col = 32h + i), one 1024-el transpose.
        u2_nm = st.tile([128, 1024], bf, tag="u2_nm")
        u2i = u2_f[:].rearrange("p (h nl) -> p h nl", h=32, nl=32)
        u2o = u2_nm[:].rearrange("p (h i) -> p h i", h=32, i=32)
        nc.vector.transpose(out=u2o[:], in_=u2i[:])

        # zpre2 = P z1[:, q] + Q u2 (NM quarter), evict bf16
        z1v = z1_nm[:].rearrange("p (t h) -> p t h", h=32)
        u2_mv = u2_nm[:].rearrange("p (h i) -> p i h", h=32, i=32)
        zpre2 = ps.tile([128, 512], fp, tag="big")
        nc.tensor.matmul(zpre2[:], pmat, z1v[:, ds(toff, TQ), :], start=True, stop=False)
        nc.tensor.matmul(zpre2[:], qmat, u2_mv[:, 0:16, :], start=False, stop=True)

        zq_nm = st.tile([128, 512], bf, tag="zq_nm")
        nc.scalar.activation(zq_nm[:], zpre2[:], AF.Identity)

        # NM -> FD, Heff2 matmul + tanh, W2 matmul + bias, FD -> NM, DMA out
        zq_fd = st.tile([128, 512], bf, tag="zq_fd")
        nc.vector.transpose(out=zq_fd[:], in_=zq_nm[:])

        pre2 = ps.tile([128, 512], fp, tag="big")
        nc.tensor.matmul(pre2[:], hi4_2, zq_fd[:], start=True, stop=True)
        h2_fd = st.tile([128, 512], bf, tag="h2_fd")
        nc.scalar.activation(h2_fd[:], pre2[:], AF.Tanh)

        opre = ps.tile([128, 512], fp, tag="big")
        nc.tensor.matmul(opre[:], w2i4, h2_fd[:], start=True, stop=True)
        out_fd = st.tile([128, 512], fp, tag="out_fd")
        nc.scalar.activation(out_fd[:], opre[:], AF.Identity, bias=bias_s[:, 1:2])

        out_nm = st.tile([128, 512], fp, tag="out_nm")
        nc.vector.transpose(out=out_nm[:], in_=out_fd[:])

        onv = out_nm[:].rearrange("p (i j2) -> p i j2", j2=32)
        ov = outb.ap().rearrange("(i n) j -> n i j", n=128)
        nc.sync.dma_start(ov, onv[:, :, 0:FOUT])

    nc.compile()
    return nc


def _host_weights(Adj_t, Adj_s, s, H, W1, b1, W2, b2):
    import ml_dtypes

    f4 = np.float32
    bf = ml_dtypes.bfloat16
    I4 = np.eye(4, dtype=f4)
    I128 = np.eye(128, dtype=f4)
    Heff = H.sum(axis=1).astype(f4)  # [2, 32, 32]

    P = (s[0] * I128 + s[1] * Adj_s).astype(f4)
    Q = (s[2] * I128 + s[3] * Adj_s).astype(f4)

    W1H = (W1 @ Heff[0]).astype(f4)
    b1H = (b1 @ Heff[0]).astype(f4)

    hi4_2 = np.kron(I4, Heff[1])
    w2pad = np.zeros((32, 32), dtype=f4)
    w2pad[:, :FOUT] = W2
    w2i4 = np.kron(I4, w2pad)

    bias2 = np.zeros((128, 2), dtype=f4)
    bias2[:, 0] = np.tile(b1H, 4)
    b2pad = np.zeros(32, dtype=f4)
    b2pad[:FOUT] = b2
    bias2[:, 1] = np.tile(b2pad, 4)

    wpk = np.zeros((NQ, 128, WPK_COLS), dtype=bf)
    for c in range(2):
        for cp in range(2):
            blk = np.kron(I4, Adj_t[32 * c : 32 * (c + 1), 32 * cp : 32 * (cp + 1)].astype(f4))
            wpk[:, :, 128 * (2 * c + cp) : 128 * (2 * c + cp + 1)] = blk.astype(bf)
    for q in range(NQ):
        for c in range(2):
            blk = np.zeros((32, 32), dtype=f4)
            blk[:, :TQ] = Adj_t[32 * c : 32 * (c + 1), TQ * q : TQ * (q + 1)]
            wpk[q, :, 128 * (4 + c) : 128 * (5 + c)] = np.kron(I4, blk).astype(bf)
    wpk[:, :, 128 * 6 : 128 * 7] = P.astype(bf)
    wpk[:, :, 128 * 7 : 128 * 8] = Q.astype(bf)
    wpk[:, :, 128 * 8 : 128 * 9] = hi4_2.astype(bf)
    wpk[:, :, 128 * 9 : 128 * 10] = w2i4.astype(bf)
    wpk[:, :, 128 * 10 : 128 * 11] = np.kron(I4, W1H).astype(bf)

    return bias2, wpk


def _in_maps(inputs):
    f4 = np.float32
    x = np.ascontiguousarray(np.asarray(inputs["x"], dtype=f4))
    bias2, wpk = _host_weights(
        np.asarray(inputs["Adj_t"], dtype=f4),
        np.asarray(inputs["Adj_s"], dtype=f4),
        np.asarray(inputs["s"], dtype=f4),
        np.asarray(inputs["H"], dtype=f4),
        np.asarray(inputs["W1"], dtype=f4),
        np.asarray(inputs["b1"], dtype=f4),
        np.asarray(inputs["W2"], dtype=f4),
        np.asarray(inputs["b2"], dtype=f4),
    )
    maps = []
    for c in range(NCORES):
        b, q = c // NQ, c % NQ
        maps.append(
            {
                "xb": np.ascontiguousarray(x[b]),
                "bias2": bias2,
                "wpk": np.ascontiguousarray(wpk[q]),
            }
        )
    return maps


def kernel(**inputs) -> np.ndarray:
    from concourse import bass_utils

    if "nc" not in _CACHE:
        _CACHE["nc"] = _build_nc()
    nc = _CACHE["nc"]

    maps = _in_maps(inputs)
    import os

    trace = bool(int(os.environ.get("GTCNN_TRACE", "0")))
    res = bass_utils.run_bass_kernel_spmd(
        nc,
        maps,
        core_ids=list(range(NCORES)),
        trace=trace,
        trace_cores=list(range(NCORES)) if trace else None,
        stitch_traces=False,
    )
    _CACHE["last_results"] = res

    out = np.empty((B, M, FOUT), dtype=np.float32)
    for c in range(NCORES):
        b, q = c // NQ, c % NQ
        out[b, 2048 * q : 2048 * (q + 1), :] = res.results[c]["outb"]
    return out


# revision 21
# speedup vs baseline: 1.3330x; 1.0711x over previous
"""Trainium2 Bass kernel for nn_GTCNN (product-graph GTCNN, 2 layers, K collapsed).

Math (per batch b, x: [M=8192, 32]):
  Adj = s0*I + s1*kron(I_t, As) + s2*kron(At, I_s) + s3*kron(At, As),  T=64, N=128
  h0 = x @ W1 + b1
  h_{l+1} = tanh((Adj @ h_l) @ Heff_l),   Heff_l = sum_k H[l, k]
  out = h2 @ W2 + b2

Device algorithm (Heff commutes with Adj, so Heff1 folds into W1 host-side):
  w  = x @ (W1 Heff1) + 1 (x) (b1 Heff1)          [FD matmul]
  z1 = tanh(P w + Q At-mix(w))                     [layer 1, all t]
  y  = P z1[:, q] + Q At[q,:]-mix(z1)              [layer 2, t-quarter]
  z2 = tanh(y @ Heff2);  out = z2 @ W2 + b2
  with P = s0*I + s1*As, Q = s2*I + s3*As folded on host.

Sharding: core c -> (b = c // 4, t-quarter q = c % 4). Layer 1 computed fully per
b (4x redundant; collectives have a ~10us floor, far above the redundant work).

Layouts (n = 32*nh + nl, t = 32*c + tl):
  NM  [n, t*32 + h]                      node-on-partition (P / Q matmuls)
  FD  [32*nh + h,  t*32 + nl]            feature-on-partition (W/Heff stationaries
                                         are block-diag kron(I4, W))
  FDT [32*nh + tl, ...]                  t-on-partition (At matmuls, stationaries
                                         kron(I4, At 32x32 block), PSUM-accum c)
All matmuls bf16 (PSUM fp32); PSUM evictions are scalar-engine copies casting to
bf16; layout moves are DVE 32x32 StreamTranspose ops on bf16 SBUF tiles. Every
transpose WRITES with stride-1 within-block (strided DVE writes cost ~3.6x);
consuming matmuls absorb the resulting layout via strided moving-AP views.
Tiles that land transposed-conventions: u_nm col = 64h+32cp+tl, g2 col =
1024c+32h+nl, u2_nm col = 32h+i.
"""

import numpy as np

T, NS, B, FIN, HID, FOUT = 64, 128, 2, 32, 32, 16
M = T * NS
NCORES, NQ = 8, 4
TQ = T // NQ  # 16 t's per quarter

_CACHE = {}

# bf16 weight pack [128, 1408]; 128-col slot i: atbd[2c+cp] i=0..3, atbq[c]
# i=4..5, P i=6, Q i=7, hi4_2 i=8, w2i4 i=9, w1hi4 i=10.
WPK_COLS = 1408


def _build_nc():
    from contextlib import ExitStack

    import concourse.mybir as mybir
    import concourse.tile as tile
    from concourse import bacc
    from concourse.bass import ds

    fp = mybir.dt.float32
    bf = mybir.dt.bfloat16
    AF = mybir.ActivationFunctionType

    nc = bacc.Bacc(
        "TRN2",
        target_bir_lowering=False,
        debug=False,
        enable_asserts=False,
        num_devices=NCORES,
    )

    xb = nc.dram_tensor("xb", [M, FIN], fp, kind="ExternalInput")
    bias2 = nc.dram_tensor("bias2", [128, 2], fp, kind="ExternalInput")
    wpk = nc.dram_tensor("wpk", [128, WPK_COLS], bf, kind="ExternalInput")
    outb = nc.dram_tensor("outb", [TQ * NS, FOUT], fp, kind="ExternalOutput")

    C512 = [slice(512 * j, 512 * (j + 1)) for j in range(4)]
    C1024 = [slice(1024 * j, 1024 * (j + 1)) for j in range(2)]

    with tile.TileContext(nc) as tc, ExitStack() as ctx:
        const = ctx.enter_context(tc.tile_pool(name="const", bufs=1))
        st = ctx.enter_context(tc.tile_pool(name="st", bufs=1))
        ps = ctx.enter_context(tc.tile_pool(name="ps", bufs=2, space="PSUM"))


        # ---- PE warm-up on an uninitialized tile: no input deps, so these
        # run at t~0 while DMAs stream, releasing the HAM clock-gate (PE is
        # ~2x slower until ~4us of sustained activity). Output never read.
        junk = const.tile([128, 512], bf, tag="junk")
        nc.gpsimd.memset(junk[:], 0)
        warm_ps = ps.tile([128, 512], fp, tag="big")
        for _ in range(14):
            nc.tensor.matmul(warm_ps[:], junk[:, 0:128], junk[:], start=True, stop=True)

        # ---- x load first (the critical path): NM [n, (t, f)], 4 t-chunks
        # split across the two HWDGE rings (sync + scalar queues).
        wpk_s = const.tile([128, WPK_COLS], bf, tag="wpk")
        nc.gpsimd.dma_start(wpk_s[:], wpk.ap())
        bias_s = const.tile([128, 2], fp, tag="bias")
        nc.gpsimd.dma_start(bias_s[:], bias2.ap())

        x_nm = st.tile([128, 2048], fp, tag="x_nm")
        xv = xb.ap().rearrange("(t n) f -> n t f", n=128)
        x_nm_v = x_nm[:].rearrange("p (t f) -> p t f", f=32)
        for j in range(8):
            eng = nc.sync if j % 2 == 0 else nc.scalar
            eng.dma_start(x_nm_v[:, 8 * j : 8 * (j + 1), :], xv[:, 8 * j : 8 * (j + 1), :])
        wslot = wpk_s[:].rearrange("p (i c) -> p i c", c=128)
        pmat = wslot[:, 6, :]
        qmat = wslot[:, 7, :]
        hi4_2 = wslot[:, 8, :]
        w2i4 = wslot[:, 9, :]
        w1hi4 = wslot[:, 10, :]

        # ---- per chunk: cast x to bf16 (ACT), NM -> FD (DVE) ----
        x_nmb = st.tile([128, 2048], bf, tag="x_nmb")
        for j in range(4):
            nc.scalar.activation(x_nmb[:, C512[j]], x_nm[:, C512[j]], AF.Identity)
        x_fd = st.tile([128, 2048], bf, tag="x_fd")
        for j in range(4):
            nc.vector.transpose(out=x_fd[:, C512[j]], in_=x_nmb[:, C512[j]])

        def pe_keepalive(k):
            # Dep-free LDWEIGHTS on the junk tile: occupies the otherwise-idle
            # PE between matmul stages so the HAM clock-gate stays released.
            for _ in range(k):
                nc.tensor.ldweights(junk[:, 0:128])

        # ---- w = x @ W1H + b1H  (FD), ACT bias-evict to bf16 ----
        wpre = ps.tile([128, 2048], fp, tag="big")
        for j in range(4):
            nc.tensor.matmul(wpre[:, C512[j]], w1hi4, x_fd[:, C512[j]], start=True, stop=True)
        pe_keepalive(10)
        w_fd = st.tile([128, 2048], bf, tag="w_fd")
        for j in range(4):
            nc.scalar.activation(w_fd[:, C512[j]], wpre[:, C512[j]], AF.Identity, bias=bias_s[:, 0:1])

        # ---- w -> NM (std) and w -> FDT (std g convention: 1024c+32nl+h) ----
        w_nm = st.tile([128, 2048], bf, tag="w_nm")
        for j in range(2):
            nc.vector.transpose(out=w_nm[:, C1024[j]], in_=w_fd[:, C1024[j]])

        g1 = st.tile([128, 2048], bf, tag="g1")
        gi = w_fd[:].rearrange("p (c tl nl) -> p c nl tl", c=2, tl=32, nl=32)
        go = g1[:].rearrange("p (c nl h) -> p c nl h", c=2, nl=32, h=32)
        for c in range(2):
            nc.vector.transpose(out=go[:, c], in_=gi[:, c])

        # ---- u1 = At-mix(w) (FDT out, PSUM-accum over c) ----
        u_ps = ps.tile([128, 2048], fp, tag="big")
        g1r = g1[:].rearrange("p (c nl h) -> p c h nl", c=2, nl=32, h=32)
        for cp in range(2):
            for hh in range(2):
                for c in range(2):
                    nc.tensor.matmul(
                        u_ps[:, cp * 1024 + 512 * hh : cp * 1024 + 512 * (hh + 1)],
                        wslot[:, 2 * c + cp, :],
                        g1r[:, c, 16 * hh : 16 * (hh + 1), :],
                        start=(c == 0),
                        stop=(c == 1),
                    )

        # ---- ACT-evict u (cast bf16), FDT -> NM on DVE (stride-1 writes:
        # u_nm stored t-inner, col = 64h + 32cp + tl) ----
        u_fdt = st.tile([128, 2048], bf, tag="u_fdt")
        for j in range(2):
            nc.scalar.activation(u_fdt[:, C1024[j]], u_ps[:, C1024[j]], AF.Identity)

        u_nm = st.tile([128, 2048], bf, tag="u_nm")
        ui = u_fdt[:].rearrange("p (cp h nl) -> p cp h nl", cp=2, h=32, nl=32)
        uo = u_nm[:].rearrange("p (h cp tl) -> p cp h tl", h=32, cp=2, tl=32)
        for cp in range(2):
            nc.vector.transpose(out=uo[:, cp], in_=ui[:, cp])

        # ---- zpre1 = P w + Q u (NM), tanh-evict -> z1_nm bf16 ----
        pe_keepalive(8)
        u_mv = u_nm[:].rearrange("p (h cp tl) -> p cp tl h", h=32, cp=2, tl=32)
        zpre = ps.tile([128, 2048], fp, tag="big")
        for j in range(4):
            nc.tensor.matmul(zpre[:, C512[j]], pmat, w_nm[:, C512[j]], start=True, stop=False)
        pe_keepalive(6)
        for j in range(4):
            nc.tensor.matmul(
                zpre[:, C512[j]],
                qmat,
                u_mv[:, j // 2, 16 * (j % 2) : 16 * (j % 2) + 16, :],
                start=False,
                stop=True,
            )

        z1_nm = st.tile([128, 2048], bf, tag="z1_nm")
        for j in range(2):
            nc.scalar.activation(z1_nm[:, C1024[j]], zpre[:, C1024[j]], AF.Tanh)

        # GPSIMD (idle) extracts this core's t-quarter of z1 so the layer-2
        # P-matmul gets a register-free moving AP (register APs on the PE cost
        # ~1.7us in TENSOR_LOADs right on the layer-2 critical path).
        pidg = nc.gpsimd.partition_id()
        toffg = (pidg % NQ) * TQ
        z1v = z1_nm[:].rearrange("p (t h) -> p t h", h=32)
        zq_cp = st.tile([128, 512], bf, tag="zq_cp")
        nc.gpsimd.tensor_copy(zq_cp[:], z1v[:, ds(toffg, TQ), :])

        # ====================== layer 2 (t-quarter only) ======================
        # g2 = FDT'(z1), stored nl-inner: col = 1024c + 32h + nl.
        g2 = st.tile([128, 2048], bf, tag="g2")
        zi = z1_nm[:].rearrange("p (c tl h) -> p c h tl", c=2, tl=32, h=32)
        zo = g2[:].rearrange("p (c h nl) -> p c h nl", c=2, h=32, nl=32)
        for c in range(2):
            nc.vector.transpose(out=zo[:, c], in_=zi[:, c])

        # zpre2 P-part first: runs on the PE while the u2 path's evict and
        # transpose are still in flight.
        zpre2 = ps.tile([128, 512], fp, tag="big")
        nc.tensor.matmul(zpre2[:], pmat, zq_cp[:], start=True, stop=False)

        # u2 = At[q rows]-mix(z1): out partitions (nh, tl' in 0..15)
        u2_ps = ps.tile([128, 1024], fp, tag="big")
        g2r = g2[:].rearrange("p (c h nl) -> p c h nl", c=2, h=32, nl=32)
        for hh in range(2):
            for c in range(2):
                nc.tensor.matmul(
                    u2_ps[:, 512 * hh : 512 * (hh + 1)],
                    wslot[:, 4 + c, :],
                    g2r[:, c, 16 * hh : 16 * (hh + 1), :],
                    start=(c == 0),
                    stop=(c == 1),
                )

        u2_f = st.tile([128, 1024], bf, tag="u2_f")
        nc.scalar.activation(u2_f[:], u2_ps[:], AF.Identity)

        # u2_nm stored i-inner (col = 32h + i), one 1024-el transpose.
        u2_nm = st.tile([128, 1024], bf, tag="u2_nm")
        u2i = u2_f[:].rearrange("p (h nl) -> p h nl", h=32, nl=32)
        u2o = u2_nm[:].rearrange("p (h i) -> p h i", h=32, i=32)
        nc.vector.transpose(out=u2o[:], in_=u2i[:])

        # zpre2 = P zq + Q u2 (NM quarter), evict bf16
        u2_mv = u2_nm[:].rearrange("p (h i) -> p i h", h=32, i=32)
        nc.tensor.matmul(zpre2[:], qmat, u2_mv[:, 0:16, :], start=False, stop=True)

        zq_nm = st.tile([128, 512], bf, tag="zq_nm")
        nc.scalar.activation(zq_nm[:], zpre2[:], AF.Identity)

        # NM -> FD, Heff2 matmul + tanh, W2 matmul + bias, FD -> NM, DMA out
        zq_fd = st.tile([128, 512], bf, tag="zq_fd")
        nc.vector.transpose(out=zq_fd[:], in_=zq_nm[:])

        pre2 = ps.tile([128, 512], fp, tag="big")
        nc.tensor.matmul(pre2[:], hi4_2, zq_fd[:], start=True, stop=True)
        h2_fd = st.tile([128, 512], bf, tag="h2_fd")
        nc.scalar.activation(h2_fd[:], pre2[:], AF.Tanh)

        opre = ps.tile([128, 512], fp, tag="big")
        nc.tensor.matmul(opre[:], w2i4, h2_fd[:], start=True, stop=True)
        out_fd = st.tile([128, 512], fp, tag="out_fd")
        nc.scalar.activation(out_fd[:], opre[:], AF.Identity, bias=bias_s[:, 1:2])

        out_nm = st.tile([128, 512], fp, tag="out_nm")
        nc.vector.transpose(out=out_nm[:], in_=out_fd[:])

        onv = out_nm[:].rearrange("p (i j2) -> p i j2", j2=32)
        ov = outb.ap().rearrange("(i n) j -> n i j", n=128)
        nc.sync.dma_start(ov, onv[:, :, 0:FOUT])

    nc.compile()
    return nc


def _host_weights(Adj_t, Adj_s, s, H, W1, b1, W2, b2):
    import ml_dtypes

    f4 = np.float32
    bf = ml_dtypes.bfloat16
    I4 = np.eye(4, dtype=f4)
    I128 = np.eye(128, dtype=f4)
    Heff = H.sum(axis=1).astype(f4)  # [2, 32, 32]

    P = (s[0] * I128 + s[1] * Adj_s).astype(f4)
    Q = (s[2] * I128 + s[3] * Adj_s).astype(f4)

    W1H = (W1 @ Heff[0]).astype(f4)
    b1H = (b1 @ Heff[0]).astype(f4)

    hi4_2 = np.kron(I4, Heff[1])
    w2pad = np.zeros((32, 32), dtype=f4)
    w2pad[:, :FOUT] = W2
    w2i4 = np.kron(I4, w2pad)

    bias2 = np.zeros((128, 2), dtype=f4)
    bias2[:, 0] = np.tile(b1H, 4)
    b2pad = np.zeros(32, dtype=f4)
    b2pad[:FOUT] = b2
    bias2[:, 1] = np.tile(b2pad, 4)

    wpk = np.zeros((NQ, 128, WPK_COLS), dtype=bf)
    for c in range(2):
        for cp in range(2):
            blk = np.kron(I4, Adj_t[32 * c : 32 * (c + 1), 32 * cp : 32 * (cp + 1)].astype(f4))
            wpk[:, :, 128 * (2 * c + cp) : 128 * (2 * c + cp + 1)] = blk.astype(bf)
    for q in range(NQ):
        for c in range(2):
            blk = np.zeros((32, 32), dtype=f4)
            blk[:, :TQ] = Adj_t[32 * c : 32 * (c + 1), TQ * q : TQ * (q + 1)]
            wpk[q, :, 128 * (4 + c) : 128 * (5 + c)] = np.kron(I4, blk).astype(bf)
    wpk[:, :, 128 * 6 : 128 * 7] = P.astype(bf)
    wpk[:, :, 128 * 7 : 128 * 8] = Q.astype(bf)
    wpk[:, :, 128 * 8 : 128 * 9] = hi4_2.astype(bf)
    wpk[:, :, 128 * 9 : 128 * 10] = w2i4.astype(bf)
    wpk[:, :, 128 * 10 : 128 * 11] = np.kron(I4, W1H).astype(bf)

    return bias2, wpk


def _in_maps(inputs):
    f4 = np.float32
    x = np.ascontiguousarray(np.asarray(inputs["x"], dtype=f4))
    bias2, wpk = _host_weights(
        np.asarray(inputs["Adj_t"], dtype=f4),
        np.asarray(inputs["Adj_s"], dtype=f4),
        np.asarray(inputs["s"], dtype=f4),
        np.asarray(inputs["H"], dtype=f4),
        np.asarray(inputs["W1"], dtype=f4),
        np.asarray(inputs["b1"], dtype=f4),
        np.asarray(inputs["W2"], dtype=f4),
        np.asarray(inputs["b2"], dtype=f4),
    )
    maps = []
    for c in range(NCORES):
        b, q = c // NQ, c % NQ
        maps.append(
            {
                "xb": np.ascontiguousarray(x[b]),
                "bias2": bias2,
                "wpk": np.ascontiguousarray(wpk[q]),
            }
        )
    return maps


def kernel(**inputs) -> np.ndarray:
    from concourse import bass_utils

    if "nc" not in _CACHE:
        _CACHE["nc"] = _build_nc()
    nc = _CACHE["nc"]

    maps = _in_maps(inputs)
    import os

    trace = bool(int(os.environ.get("GTCNN_TRACE", "0")))
    res = bass_utils.run_bass_kernel_spmd(
        nc,
        maps,
        core_ids=list(range(NCORES)),
        trace=trace,
        trace_cores=list(range(NCORES)) if trace else None,
        stitch_traces=False,
    )
    _CACHE["last_results"] = res

    out = np.empty((B, M, FOUT), dtype=np.float32)
    for c in range(NCORES):
        b, q = c // NQ, c % NQ
        out[b, 2048 * q : 2048 * (q + 1), :] = res.results[c]["outb"]
    return out


# revision 23
# speedup vs baseline: 1.3587x; 1.0193x over previous
"""Trainium2 Bass kernel for nn_GTCNN (product-graph GTCNN, 2 layers, K collapsed).

Math (per batch b, x: [M=8192, 32]):
  Adj = s0*I + s1*kron(I_t, As) + s2*kron(At, I_s) + s3*kron(At, As),  T=64, N=128
  h0 = x @ W1 + b1
  h_{l+1} = tanh((Adj @ h_l) @ Heff_l),   Heff_l = sum_k H[l, k]
  out = h2 @ W2 + b2

Device algorithm (Heff commutes with Adj, so Heff1 folds into W1 host-side):
  w  = x @ (W1 Heff1) + 1 (x) (b1 Heff1)          [FD matmul]
  z1 = tanh(P w + Q At-mix(w))                     [layer 1, all t]
  y  = P z1[:, q] + Q At[q,:]-mix(z1)              [layer 2, t-quarter]
  z2 = tanh(y @ Heff2);  out = z2 @ W2 + b2
  with P = s0*I + s1*As, Q = s2*I + s3*As folded on host.

Sharding: core c -> (b = c // 4, t-quarter q = c % 4). Layer 1 computed fully per
b (4x redundant; collectives have a ~10us floor, far above the redundant work).

Layouts (n = 32*nh + nl, t = 32*c + tl):
  NM  [n, t*32 + h]                      node-on-partition (P / Q matmuls)
  FD  [32*nh + h,  t*32 + nl]            feature-on-partition (W/Heff stationaries
                                         are block-diag kron(I4, W))
  FDT [32*nh + tl, ...]                  t-on-partition (At matmuls, stationaries
                                         kron(I4, At 32x32 block), PSUM-accum c)
All matmuls bf16 (PSUM fp32); PSUM evictions are scalar-engine copies casting to
bf16; layout moves are DVE 32x32 StreamTranspose ops on bf16 SBUF tiles. Every
transpose WRITES with stride-1 within-block (strided DVE writes cost ~3.6x);
consuming matmuls absorb the resulting layout via strided moving-AP views.
Tiles that land transposed-conventions: u_nm col = 64h+32cp+tl, g2 col =
1024c+32h+nl, u2_nm col = 32h+i.
"""

import numpy as np

T, NS, B, FIN, HID, FOUT = 64, 128, 2, 32, 32, 16
M = T * NS
NCORES, NQ = 8, 4
TQ = T // NQ  # 16 t's per quarter

_CACHE = {}

# bf16 weight pack [128, 1408]; 128-col slot i: atbd[2c+cp] i=0..3, atbq[c]
# i=4..5, P i=6, Q i=7, hi4_2 i=8, w2i4 i=9, w1hi4 i=10.
WPK_COLS = 1408


def _build_nc():
    from contextlib import ExitStack

    import concourse.mybir as mybir
    import concourse.tile as tile
    from concourse import bacc
    from concourse.bass import ds

    fp = mybir.dt.float32
    bf = mybir.dt.bfloat16
    AF = mybir.ActivationFunctionType

    nc = bacc.Bacc(
        "TRN2",
        target_bir_lowering=False,
        debug=False,
        enable_asserts=False,
        num_devices=NCORES,
    )

    xb = nc.dram_tensor("xb", [M, FIN], fp, kind="ExternalInput")
    w1h = nc.dram_tensor("w1h", [128, 128], fp, kind="ExternalInput")
    bias2 = nc.dram_tensor("bias2", [128, 2], fp, kind="ExternalInput")
    wpk = nc.dram_tensor("wpk", [128, WPK_COLS], bf, kind="ExternalInput")
    outb = nc.dram_tensor("outb", [TQ * NS, FOUT], fp, kind="ExternalOutput")

    C512 = [slice(512 * j, 512 * (j + 1)) for j in range(4)]
    C1024 = [slice(1024 * j, 1024 * (j + 1)) for j in range(2)]

    with tile.TileContext(nc) as tc, ExitStack() as ctx:
        const = ctx.enter_context(tc.tile_pool(name="const", bufs=1))
        st = ctx.enter_context(tc.tile_pool(name="st", bufs=1))
        ps = ctx.enter_context(tc.tile_pool(name="ps", bufs=2, space="PSUM"))


        # ---- PE warm-up on an uninitialized tile: no input deps, so these
        # run at t~0 while DMAs stream, releasing the HAM clock-gate (PE is
        # ~2x slower until ~4us of sustained activity). Output never read.
        junk = const.tile([128, 512], bf, tag="junk")
        nc.gpsimd.memset(junk[:], 0)
        warm_ps = ps.tile([128, 512], fp, tag="big")
        for _ in range(14):
            nc.tensor.matmul(warm_ps[:], junk[:, 0:128], junk[:], start=True, stop=True)

        # ---- x load first (the critical path): NM [n, (t, f)], 4 t-chunks
        # split across the two HWDGE rings (sync + scalar queues).
        wpk_s = const.tile([128, WPK_COLS], bf, tag="wpk")
        nc.gpsimd.dma_start(wpk_s[:], wpk.ap())
        w1h_s = const.tile([128, 128], fp, tag="w1h")
        nc.gpsimd.dma_start(w1h_s[:], w1h.ap())
        bias_s = const.tile([128, 2], fp, tag="bias")
        nc.gpsimd.dma_start(bias_s[:], bias2.ap())

        x_nm = st.tile([128, 2048], fp, tag="x_nm")
        xv = xb.ap().rearrange("(t n) f -> n t f", n=128)
        x_nm_v = x_nm[:].rearrange("p (t f) -> p t f", f=32)
        for j in range(8):
            eng = nc.sync if j % 2 == 0 else nc.scalar
            eng.dma_start(x_nm_v[:, 8 * j : 8 * (j + 1), :], xv[:, 8 * j : 8 * (j + 1), :])
        wslot = wpk_s[:].rearrange("p (i c) -> p i c", c=128)
        pmat = wslot[:, 6, :]
        qmat = wslot[:, 7, :]
        hi4_2 = wslot[:, 8, :]
        w2i4 = wslot[:, 9, :]

        # ---- per chunk: NM -> FD (DVE, fp32; casting on ACT would put the
        # x-DMA issue queue in front of the casts and stall the ladder) ----
        x_fd = st.tile([128, 2048], fp, tag="x_fd")
        for j in range(4):
            nc.vector.transpose(out=x_fd[:, C512[j]], in_=x_nm[:, C512[j]])

        def pe_keepalive(k):
            # Dep-free LDWEIGHTS on the junk tile: occupies the otherwise-idle
            # PE between matmul stages so the HAM clock-gate stays released.
            for _ in range(k):
                nc.tensor.ldweights(junk[:, 0:128])

        # ---- w = x @ W1H + b1H  (FD, fp32 matmuls), ACT bias-evict bf16.
        # The whole layer-1 midsection is interleaved at t-half (c) granularity
        # so the PE never idles long enough to re-engage the HAM throttle:
        # evict[c] -> {w_nm[c], g1[c]} -> u1 mms for contraction-half c ->
        # P mms for output-half c, with the u eviction/transpose and Q mms
        # trailing one half behind.
        wpre = ps.tile([128, 2048], fp, tag="big")
        for j in range(4):
            nc.tensor.matmul(wpre[:, C512[j]], w1h_s[:], x_fd[:, C512[j]], start=True, stop=True)
        pe_keepalive(16)

        w_fd = st.tile([128, 2048], bf, tag="w_fd")
        w_nm = st.tile([128, 2048], bf, tag="w_nm")
        g1 = st.tile([128, 2048], bf, tag="g1")
        u_ps = ps.tile([128, 2048], fp, tag="big")
        gi = w_fd[:].rearrange("p (c tl nl) -> p c nl tl", c=2, tl=32, nl=32)
        go = g1[:].rearrange("p (c nl h) -> p c nl h", c=2, nl=32, h=32)
        g1r = g1[:].rearrange("p (c nl h) -> p c h nl", c=2, nl=32, h=32)
        zpre = None

        for c in range(2):
            nc.scalar.activation(
                w_fd[:, C1024[c]], wpre[:, C1024[c]], AF.Identity, bias=bias_s[:, 0:1]
            )
            nc.vector.transpose(out=go[:, c], in_=gi[:, c])
            nc.vector.transpose(out=w_nm[:, C1024[c]], in_=w_fd[:, C1024[c]])
            # u1 contributions from contraction-half c (both output halves)
            for cp in range(2):
                for hh in range(2):
                    nc.tensor.matmul(
                        u_ps[:, cp * 1024 + 512 * hh : cp * 1024 + 512 * (hh + 1)],
                        wslot[:, 2 * c + cp, :],
                        g1r[:, c, 16 * hh : 16 * (hh + 1), :],
                        start=(c == 0),
                        stop=(c == 1),
                    )
            # P mms for output chunks of this half (needs only w_nm[c])
            if c == 0:
                zpre = ps.tile([128, 2048], fp, tag="big")
            for j in (2 * c, 2 * c + 1):
                nc.tensor.matmul(zpre[:, C512[j]], pmat, w_nm[:, C512[j]], start=True, stop=False)

        # ---- ACT-evict u (cast bf16), FDT -> NM on DVE (u_nm t-inner:
        # col = 64h + 32cp + tl, so the transpose writes stride-1) ----
        u_fdt = st.tile([128, 2048], bf, tag="u_fdt")
        u_nm = st.tile([128, 2048], bf, tag="u_nm")
        ui = u_fdt[:].rearrange("p (cp h nl) -> p cp h nl", cp=2, h=32, nl=32)
        uo = u_nm[:].rearrange("p (h cp tl) -> p cp h tl", h=32, cp=2, tl=32)
        u_mv = u_nm[:].rearrange("p (h cp tl) -> p cp tl h", h=32, cp=2, tl=32)
        for cp in range(2):
            nc.scalar.activation(u_fdt[:, C1024[cp]], u_ps[:, C1024[cp]], AF.Identity)
            nc.vector.transpose(out=uo[:, cp], in_=ui[:, cp])
            for j in (2 * cp, 2 * cp + 1):
                nc.tensor.matmul(
                    zpre[:, C512[j]],
                    qmat,
                    u_mv[:, cp, 16 * (j % 2) : 16 * (j % 2) + 16, :],
                    start=False,
                    stop=True,
                )

        z1_nm = st.tile([128, 2048], bf, tag="z1_nm")
        for j in range(2):
            nc.scalar.activation(z1_nm[:, C1024[j]], zpre[:, C1024[j]], AF.Tanh)

        # GPSIMD (idle) extracts this core's t-quarter of z1 so the layer-2
        # P-matmul gets a register-free moving AP (register APs on the PE cost
        # ~1.7us in TENSOR_LOADs right on the layer-2 critical path).
        pidg = nc.gpsimd.partition_id()
        toffg = (pidg % NQ) * TQ
        z1v = z1_nm[:].rearrange("p (t h) -> p t h", h=32)
        zq_cp = st.tile([128, 512], bf, tag="zq_cp")
        nc.gpsimd.tensor_copy(zq_cp[:], z1v[:, ds(toffg, TQ), :])

        # ====================== layer 2 (t-quarter only) ======================
        # g2 = FDT'(z1), stored nl-inner: col = 1024c + 32h + nl.
        g2 = st.tile([128, 2048], bf, tag="g2")
        zi = z1_nm[:].rearrange("p (c tl h) -> p c h tl", c=2, tl=32, h=32)
        zo = g2[:].rearrange("p (c h nl) -> p c h nl", c=2, h=32, nl=32)
        for c in range(2):
            nc.vector.transpose(out=zo[:, c], in_=zi[:, c])

        # zpre2 P-part first: runs on the PE while the u2 path's evict and
        # transpose are still in flight.
        zpre2 = ps.tile([128, 512], fp, tag="big")
        nc.tensor.matmul(zpre2[:], pmat, zq_cp[:], start=True, stop=False)

        # u2 = At[q rows]-mix(z1): out partitions (nh, tl' in 0..15)
        u2_ps = ps.tile([128, 1024], fp, tag="big")
        g2r = g2[:].rearrange("p (c h nl) -> p c h nl", c=2, h=32, nl=32)
        for hh in range(2):
            for c in range(2):
                nc.tensor.matmul(
                    u2_ps[:, 512 * hh : 512 * (hh + 1)],
                    wslot[:, 4 + c, :],
                    g2r[:, c, 16 * hh : 16 * (hh + 1), :],
                    start=(c == 0),
                    stop=(c == 1),
                )

        u2_f = st.tile([128, 1024], bf, tag="u2_f")
        nc.scalar.activation(u2_f[:], u2_ps[:], AF.Identity)

        # u2_nm stored i-inner (col = 32h + i), one 1024-el transpose.
        u2_nm = st.tile([128, 1024], bf, tag="u2_nm")
        u2i = u2_f[:].rearrange("p (h nl) -> p h nl", h=32, nl=32)
        u2o = u2_nm[:].rearrange("p (h i) -> p h i", h=32, i=32)
        nc.vector.transpose(out=u2o[:], in_=u2i[:])

        # zpre2 = P zq + Q u2 (NM quarter), evict bf16
        u2_mv = u2_nm[:].rearrange("p (h i) -> p i h", h=32, i=32)
        nc.tensor.matmul(zpre2[:], qmat, u2_mv[:, 0:16, :], start=False, stop=True)

        zq_nm = st.tile([128, 512], bf, tag="zq_nm")
        nc.scalar.activation(zq_nm[:], zpre2[:], AF.Identity)

        # NM -> FD, Heff2 matmul + tanh, W2 matmul + bias, FD -> NM, DMA out
        zq_fd = st.tile([128, 512], bf, tag="zq_fd")
        nc.vector.transpose(out=zq_fd[:], in_=zq_nm[:])

        pre2 = ps.tile([128, 512], fp, tag="big")
        nc.tensor.matmul(pre2[:], hi4_2, zq_fd[:], start=True, stop=True)
        h2_fd = st.tile([128, 512], bf, tag="h2_fd")
        nc.scalar.activation(h2_fd[:], pre2[:], AF.Tanh)

        opre = ps.tile([128, 512], fp, tag="big")
        nc.tensor.matmul(opre[:], w2i4, h2_fd[:], start=True, stop=True)
        out_fd = st.tile([128, 512], fp, tag="out_fd")
        nc.scalar.activation(out_fd[:], opre[:], AF.Identity, bias=bias_s[:, 1:2])

        out_nm = st.tile([128, 512], fp, tag="out_nm")
        nc.vector.transpose(out=out_nm[:], in_=out_fd[:])

        onv = out_nm[:].rearrange("p (i j2) -> p i j2", j2=32)
        ov = outb.ap().rearrange("(i n) j -> n i j", n=128)
        nc.sync.dma_start(ov, onv[:, :, 0:FOUT])

    nc.compile()
    return nc


def _host_weights(Adj_t, Adj_s, s, H, W1, b1, W2, b2):
    import ml_dtypes

    f4 = np.float32
    bf = ml_dtypes.bfloat16
    I4 = np.eye(4, dtype=f4)
    I128 = np.eye(128, dtype=f4)
    Heff = H.sum(axis=1).astype(f4)  # [2, 32, 32]

    P = (s[0] * I128 + s[1] * Adj_s).astype(f4)
    Q = (s[2] * I128 + s[3] * Adj_s).astype(f4)

    W1H = (W1 @ Heff[0]).astype(f4)
    b1H = (b1 @ Heff[0]).astype(f4)

    hi4_2 = np.kron(I4, Heff[1])
    w2pad = np.zeros((32, 32), dtype=f4)
    w2pad[:, :FOUT] = W2
    w2i4 = np.kron(I4, w2pad)

    bias2 = np.zeros((128, 2), dtype=f4)
    bias2[:, 0] = np.tile(b1H, 4)
    b2pad = np.zeros(32, dtype=f4)
    b2pad[:FOUT] = b2
    bias2[:, 1] = np.tile(b2pad, 4)

    wpk = np.zeros((NQ, 128, WPK_COLS), dtype=bf)
    for c in range(2):
        for cp in range(2):
            blk = np.kron(I4, Adj_t[32 * c : 32 * (c + 1), 32 * cp : 32 * (cp + 1)].astype(f4))
            wpk[:, :, 128 * (2 * c + cp) : 128 * (2 * c + cp + 1)] = blk.astype(bf)
    for q in range(NQ):
        for c in range(2):
            blk = np.zeros((32, 32), dtype=f4)
            blk[:, :TQ] = Adj_t[32 * c : 32 * (c + 1), TQ * q : TQ * (q + 1)]
            wpk[q, :, 128 * (4 + c) : 128 * (5 + c)] = np.kron(I4, blk).astype(bf)
    wpk[:, :, 128 * 6 : 128 * 7] = P.astype(bf)
    wpk[:, :, 128 * 7 : 128 * 8] = Q.astype(bf)
    wpk[:, :, 128 * 8 : 128 * 9] = hi4_2.astype(bf)
    wpk[:, :, 128 * 9 : 128 * 10] = w2i4.astype(bf)
    w1h = np.kron(I4, W1H)

    return w1h, bias2, wpk


def _in_maps(inputs):
    f4 = np.float32
    x = np.ascontiguousarray(np.asarray(inputs["x"], dtype=f4))
    w1h, bias2, wpk = _host_weights(
        np.asarray(inputs["Adj_t"], dtype=f4),
        np.asarray(inputs["Adj_s"], dtype=f4),
        np.asarray(inputs["s"], dtype=f4),
        np.asarray(inputs["H"], dtype=f4),
        np.asarray(inputs["W1"], dtype=f4),
        np.asarray(inputs["b1"], dtype=f4),
        np.asarray(inputs["W2"], dtype=f4),
        np.asarray(inputs["b2"], dtype=f4),
    )
    maps = []
    for c in range(NCORES):
        b, q = c // NQ, c % NQ
        maps.append(
            {
                "xb": np.ascontiguousarray(x[b]),
                "w1h": w1h,
                "bias2": bias2,
                "wpk": np.ascontiguousarray(wpk[q]),
            }
        )
    return maps


def kernel(**inputs) -> np.ndarray:
    from concourse import bass_utils

    if "nc" not in _CACHE:
        _CACHE["nc"] = _build_nc()
    nc = _CACHE["nc"]

    maps = _in_maps(inputs)
    import os

    trace = bool(int(os.environ.get("GTCNN_TRACE", "0")))
    res = bass_utils.run_bass_kernel_spmd(
        nc,
        maps,
        core_ids=list(range(NCORES)),
        trace=trace,
        trace_cores=list(range(NCORES)) if trace else None,
        stitch_traces=False,
    )
    _CACHE["last_results"] = res

    out = np.empty((B, M, FOUT), dtype=np.float32)
    for c in range(NCORES):
        b, q = c // NQ, c % NQ
        out[b, 2048 * q : 2048 * (q + 1), :] = res.results[c]["outb"]
    return out


# revision 25
# speedup vs baseline: 1.5033x; 1.1064x over previous
"""Trainium2 Bass kernel for nn_GTCNN (product-graph GTCNN, 2 layers, K collapsed).

Math (per batch b, x: [M=8192, 32]):
  Adj = s0*I + s1*kron(I_t, As) + s2*kron(At, I_s) + s3*kron(At, As),  T=64, N=128
  h0 = x @ W1 + b1
  h_{l+1} = tanh((Adj @ h_l) @ Heff_l),   Heff_l = sum_k H[l, k]
  out = h2 @ W2 + b2

Device algorithm (Heff commutes with Adj, so Heff1 folds into W1 host-side):
  w  = x @ (W1 Heff1) + 1 (x) (b1 Heff1)          [FD matmul]
  z1 = tanh(P w + Q At-mix(w))                     [layer 1, all t]
  y  = P z1[:, q] + Q At[q,:]-mix(z1)              [layer 2, t-quarter]
  z2 = tanh(y @ Heff2);  out = z2 @ W2 + b2
  with P = s0*I + s1*As, Q = s2*I + s3*As folded on host.

Sharding: core c -> (b = c // 4, t-quarter q = c % 4). Layer 1 computed fully per
b (4x redundant; collectives have a ~10us floor, far above the redundant work).

Layouts (n = 32*nh + nl, t = 32*c + tl):
  NM  [n, t*32 + h]                      node-on-partition (P / Q matmuls)
  FD  [32*nh + h,  t*32 + nl]            feature-on-partition (W/Heff stationaries
                                         are block-diag kron(I4, W))
  FDT [32*nh + tl, ...]                  t-on-partition (At matmuls, stationaries
                                         kron(I4, At 32x32 block), PSUM-accum c)
All matmuls bf16 (PSUM fp32); PSUM evictions are scalar-engine copies casting to
bf16; layout moves are DVE 32x32 StreamTranspose ops on bf16 SBUF tiles. Every
transpose WRITES with stride-1 within-block (strided DVE writes cost ~3.6x);
consuming matmuls absorb the resulting layout via strided moving-AP views.
Tiles that land transposed-conventions: u_nm col = 64h+32cp+tl, g2 col =
1024c+32h+nl, u2_nm col = 32h+i.
"""

import numpy as np

T, NS, B, FIN, HID, FOUT = 64, 128, 2, 32, 32, 16
M = T * NS
NCORES, NQ = 8, 4
TQ = T // NQ  # 16 t's per quarter

_CACHE = {}

# bf16 weight pack [128, 1408]; 128-col slot i: atbd[2c+cp] i=0..3, atbq[c]
# i=4..5, P i=6, Q i=7, hi4_2 i=8, w2i4 i=9, w1hi4 i=10.
WPK_COLS = 1408


def _build_nc():
    from contextlib import ExitStack

    import concourse.mybir as mybir
    import concourse.tile as tile
    from concourse import bacc
    from concourse.bass import ds

    fp = mybir.dt.float32
    bf = mybir.dt.bfloat16
    AF = mybir.ActivationFunctionType

    nc = bacc.Bacc(
        "TRN2",
        target_bir_lowering=False,
        debug=False,
        enable_asserts=False,
        num_devices=NCORES,
    )

    xb = nc.dram_tensor("xb", [M, FIN], fp, kind="ExternalInput")
    w1h = nc.dram_tensor("w1h", [128, 128], fp, kind="ExternalInput")
    bias2 = nc.dram_tensor("bias2", [128, 2], fp, kind="ExternalInput")
    wpk = nc.dram_tensor("wpk", [128, WPK_COLS], bf, kind="ExternalInput")
    outb = nc.dram_tensor("outb", [TQ * NS, FOUT], fp, kind="ExternalOutput")

    C512 = [slice(512 * j, 512 * (j + 1)) for j in range(4)]
    C1024 = [slice(1024 * j, 1024 * (j + 1)) for j in range(2)]

    with tile.TileContext(nc) as tc, ExitStack() as ctx:
        const = ctx.enter_context(tc.tile_pool(name="const", bufs=1))
        st = ctx.enter_context(tc.tile_pool(name="st", bufs=1))
        ps = ctx.enter_context(tc.tile_pool(name="ps", bufs=4, space="PSUM"))


        # ---- PE warm-up on an uninitialized tile: no input deps, so these
        # run at t~0 while DMAs stream, releasing the HAM clock-gate (PE is
        # ~2x slower until ~4us of sustained activity). Output never read.
        junk = const.tile([128, 512], bf, tag="junk")
        nc.gpsimd.memset(junk[:], 0)
        warm_ps = ps.tile([128, 512], fp, tag="big")
        for _ in range(14):
            nc.tensor.matmul(warm_ps[:], junk[:, 0:128], junk[:], start=True, stop=True)

        # ---- x load first (the critical path): NM [n, (t, f)], 4 t-chunks
        # split across the two HWDGE rings (sync + scalar queues).
        wpk_s = const.tile([128, WPK_COLS], bf, tag="wpk")
        nc.gpsimd.dma_start(wpk_s[:], wpk.ap())
        w1h_s = const.tile([128, 128], fp, tag="w1h")
        nc.gpsimd.dma_start(w1h_s[:], w1h.ap())
        bias_s = const.tile([128, 2], fp, tag="bias")
        nc.gpsimd.dma_start(bias_s[:], bias2.ap())

        x_nm = st.tile([128, 2048], fp, tag="x_nm")
        xv = xb.ap().rearrange("(t n) f -> n t f", n=128)
        x_nm_v = x_nm[:].rearrange("p (t f) -> p t f", f=32)
        for j in range(8):
            eng = nc.sync if j % 2 == 0 else nc.scalar
            eng.dma_start(x_nm_v[:, 8 * j : 8 * (j + 1), :], xv[:, 8 * j : 8 * (j + 1), :])
        wslot = wpk_s[:].rearrange("p (i c) -> p i c", c=128)
        pmat = wslot[:, 6, :]
        qmat = wslot[:, 7, :]
        hi4_2 = wslot[:, 8, :]
        w2i4 = wslot[:, 9, :]

        # ---- per chunk: NM -> FD (DVE, fp32; casting on ACT would put the
        # x-DMA issue queue in front of the casts and stall the ladder) ----
        x_fd = st.tile([128, 2048], fp, tag="x_fd")
        for j in range(4):
            nc.vector.transpose(out=x_fd[:, C512[j]], in_=x_nm[:, C512[j]])

        def pe_keepalive(k):
            # Dep-free LDWEIGHTS on the junk tile: occupies the otherwise-idle
            # PE between matmul stages so the HAM clock-gate stays released.
            for _ in range(k):
                nc.tensor.ldweights(junk[:, 0:128])

        # ---- w = x @ W1H + b1H  (FD, fp32 matmuls), ACT bias-evict bf16.
        # The whole layer-1 midsection is interleaved at t-half (c) granularity
        # so the PE never idles long enough to re-engage the HAM throttle:
        # evict[c] -> {w_nm[c], g1[c]} -> u1 mms for contraction-half c ->
        # P mms for output-half c, with the u eviction/transpose and Q mms
        # trailing one half behind.
        wpre_h = [ps.tile([128, 1024], fp, tag="big", name=f"wpre{c}") for c in range(2)]
        for j in range(4):
            nc.tensor.matmul(
                wpre_h[j // 2][:, 512 * (j % 2) : 512 * (j % 2) + 512],
                w1h_s[:],
                x_fd[:, C512[j]],
                start=True,
                stop=True,
            )
        pe_keepalive(16)

        w_fd = st.tile([128, 2048], bf, tag="w_fd")
        w_nm = st.tile([128, 2048], bf, tag="w_nm")
        g1 = st.tile([128, 2048], bf, tag="g1")
        u_ps_h = [ps.tile([128, 1024], fp, tag="big", name=f"ups{c}") for c in range(2)]
        gi = w_fd[:].rearrange("p (c tl nl) -> p c nl tl", c=2, tl=32, nl=32)
        go = g1[:].rearrange("p (c nl h) -> p c nl h", c=2, nl=32, h=32)
        g1r = g1[:].rearrange("p (c nl h) -> p c h nl", c=2, nl=32, h=32)
        zpre_h = [None, None]

        for c in range(2):
            nc.scalar.activation(
                w_fd[:, C1024[c]], wpre_h[c][:], AF.Identity, bias=bias_s[:, 0:1]
            )
            nc.vector.transpose(out=go[:, c], in_=gi[:, c])
            nc.vector.transpose(out=w_nm[:, C1024[c]], in_=w_fd[:, C1024[c]])
            # u1 contributions from contraction-half c (both output halves)
            for cp in range(2):
                for hh in range(2):
                    nc.tensor.matmul(
                        u_ps_h[cp][:, 512 * hh : 512 * (hh + 1)],
                        wslot[:, 2 * c + cp, :],
                        g1r[:, c, 16 * hh : 16 * (hh + 1), :],
                        start=(c == 0),
                        stop=(c == 1),
                    )
            # P mms for output chunks of this half (needs only w_nm[c])
            zpre_h[c] = ps.tile([128, 1024], fp, tag="big", name=f"zpre{c}")
            for j in (2 * c, 2 * c + 1):
                nc.tensor.matmul(
                    zpre_h[c][:, 512 * (j % 2) : 512 * (j % 2) + 512],
                    pmat,
                    w_nm[:, C512[j]],
                    start=True,
                    stop=False,
                )

        # ---- ACT-evict u (cast bf16), FDT -> NM on DVE (u_nm t-inner:
        # col = 64h + 32cp + tl, so the transpose writes stride-1) ----
        u_fdt = st.tile([128, 2048], bf, tag="u_fdt")
        u_nm = st.tile([128, 2048], bf, tag="u_nm")
        ui = u_fdt[:].rearrange("p (cp h nl) -> p cp h nl", cp=2, h=32, nl=32)
        uo = u_nm[:].rearrange("p (h cp tl) -> p cp h tl", h=32, cp=2, tl=32)
        u_mv = u_nm[:].rearrange("p (h cp tl) -> p cp tl h", h=32, cp=2, tl=32)
        for cp in range(2):
            nc.scalar.activation(u_fdt[:, C1024[cp]], u_ps_h[cp][:], AF.Identity)
            nc.vector.transpose(out=uo[:, cp], in_=ui[:, cp])
            for j in (2 * cp, 2 * cp + 1):
                nc.tensor.matmul(
                    zpre_h[cp][:, 512 * (j % 2) : 512 * (j % 2) + 512],
                    qmat,
                    u_mv[:, cp, 16 * (j % 2) : 16 * (j % 2) + 16, :],
                    start=False,
                    stop=True,
                )

        z1_nm = st.tile([128, 2048], bf, tag="z1_nm")
        for j in range(2):
            nc.scalar.activation(z1_nm[:, C1024[j]], zpre_h[j][:], AF.Tanh)

        # GPSIMD (idle) extracts this core's t-quarter of z1 so the layer-2
        # P-matmul gets a register-free moving AP (register APs on the PE cost
        # ~1.7us in TENSOR_LOADs right on the layer-2 critical path).
        pidg = nc.gpsimd.partition_id()
        toffg = (pidg % NQ) * TQ
        z1v = z1_nm[:].rearrange("p (t h) -> p t h", h=32)
        zq_cp = st.tile([128, 512], bf, tag="zq_cp")
        nc.gpsimd.tensor_copy(zq_cp[:], z1v[:, ds(toffg, TQ), :])

        # ====================== layer 2 (t-quarter only) ======================
        # g2 = FDT'(z1), stored nl-inner: col = 1024c + 32h + nl.
        g2 = st.tile([128, 2048], bf, tag="g2")
        zi = z1_nm[:].rearrange("p (c tl h) -> p c h tl", c=2, tl=32, h=32)
        zo = g2[:].rearrange("p (c h nl) -> p c h nl", c=2, h=32, nl=32)
        for c in range(2):
            nc.vector.transpose(out=zo[:, c], in_=zi[:, c])

        # zpre2 P-part first: runs on the PE while the u2 path's evict and
        # transpose are still in flight.
        zpre2 = ps.tile([128, 512], fp, tag="big")
        nc.tensor.matmul(zpre2[:], pmat, zq_cp[:], start=True, stop=False)

        # u2 = At[q rows]-mix(z1): out partitions (nh, tl' in 0..15)
        u2_ps = ps.tile([128, 1024], fp, tag="big")
        g2r = g2[:].rearrange("p (c h nl) -> p c h nl", c=2, h=32, nl=32)
        for hh in range(2):
            for c in range(2):
                nc.tensor.matmul(
                    u2_ps[:, 512 * hh : 512 * (hh + 1)],
                    wslot[:, 4 + c, :],
                    g2r[:, c, 16 * hh : 16 * (hh + 1), :],
                    start=(c == 0),
                    stop=(c == 1),
                )

        u2_f = st.tile([128, 1024], bf, tag="u2_f")
        nc.scalar.activation(u2_f[:], u2_ps[:], AF.Identity)

        # u2_nm stored i-inner (col = 32h + i), one 1024-el transpose.
        u2_nm = st.tile([128, 1024], bf, tag="u2_nm")
        u2i = u2_f[:].rearrange("p (h nl) -> p h nl", h=32, nl=32)
        u2o = u2_nm[:].rearrange("p (h i) -> p h i", h=32, i=32)
        nc.vector.transpose(out=u2o[:], in_=u2i[:])

        # zpre2 = P zq + Q u2 (NM quarter), evict bf16
        u2_mv = u2_nm[:].rearrange("p (h i) -> p i h", h=32, i=32)
        nc.tensor.matmul(zpre2[:], qmat, u2_mv[:, 0:16, :], start=False, stop=True)

        zq_nm = st.tile([128, 512], bf, tag="zq_nm")
        nc.scalar.activation(zq_nm[:], zpre2[:], AF.Identity)

        # NM -> FD, Heff2 matmul + tanh, W2 matmul + bias, FD -> NM, DMA out
        zq_fd = st.tile([128, 512], bf, tag="zq_fd")
        nc.vector.transpose(out=zq_fd[:], in_=zq_nm[:])

        pre2 = ps.tile([128, 512], fp, tag="big")
        nc.tensor.matmul(pre2[:], hi4_2, zq_fd[:], start=True, stop=True)
        h2_fd = st.tile([128, 512], bf, tag="h2_fd")
        nc.scalar.activation(h2_fd[:], pre2[:], AF.Tanh)

        opre = ps.tile([128, 512], fp, tag="big")
        nc.tensor.matmul(opre[:], w2i4, h2_fd[:], start=True, stop=True)
        out_fd = st.tile([128, 512], fp, tag="out_fd")
        nc.scalar.activation(out_fd[:], opre[:], AF.Identity, bias=bias_s[:, 1:2])

        out_nm = st.tile([128, 512], fp, tag="out_nm")
        nc.vector.transpose(out=out_nm[:], in_=out_fd[:])

        onv = out_nm[:].rearrange("p (i j2) -> p i j2", j2=32)
        ov = outb.ap().rearrange("(i n) j -> n i j", n=128)
        nc.sync.dma_start(ov, onv[:, :, 0:FOUT])

    nc.compile()
    return nc


def _host_weights(Adj_t, Adj_s, s, H, W1, b1, W2, b2):
    import ml_dtypes

    f4 = np.float32
    bf = ml_dtypes.bfloat16
    I4 = np.eye(4, dtype=f4)
    I128 = np.eye(128, dtype=f4)
    Heff = H.sum(axis=1).astype(f4)  # [2, 32, 32]

    P = (s[0] * I128 + s[1] * Adj_s).astype(f4)
    Q = (s[2] * I128 + s[3] * Adj_s).astype(f4)

    W1H = (W1 @ Heff[0]).astype(f4)
    b1H = (b1 @ Heff[0]).astype(f4)

    hi4_2 = np.kron(I4, Heff[1])
    w2pad = np.zeros((32, 32), dtype=f4)
    w2pad[:, :FOUT] = W2
    w2i4 = np.kron(I4, w2pad)

    bias2 = np.zeros((128, 2), dtype=f4)
    bias2[:, 0] = np.tile(b1H, 4)
    b2pad = np.zeros(32, dtype=f4)
    b2pad[:FOUT] = b2
    bias2[:, 1] = np.tile(b2pad, 4)

    wpk = np.zeros((NQ, 128, WPK_COLS), dtype=bf)
    for c in range(2):
        for cp in range(2):
            blk = np.kron(I4, Adj_t[32 * c : 32 * (c + 1), 32 * cp : 32 * (cp + 1)].astype(f4))
            wpk[:, :, 128 * (2 * c + cp) : 128 * (2 * c + cp + 1)] = blk.astype(bf)
    for q in range(NQ):
        for c in range(2):
            blk = np.zeros((32, 32), dtype=f4)
            blk[:, :TQ] = Adj_t[32 * c : 32 * (c + 1), TQ * q : TQ * (q + 1)]
            wpk[q, :, 128 * (4 + c) : 128 * (5 + c)] = np.kron(I4, blk).astype(bf)
    wpk[:, :, 128 * 6 : 128 * 7] = P.astype(bf)
    wpk[:, :, 128 * 7 : 128 * 8] = Q.astype(bf)
    wpk[:, :, 128 * 8 : 128 * 9] = hi4_2.astype(bf)
    wpk[:, :, 128 * 9 : 128 * 10] = w2i4.astype(bf)
    w1h = np.kron(I4, W1H)

    return w1h, bias2, wpk


def _in_maps(inputs):
    f4 = np.float32
    x = np.ascontiguousarray(np.asarray(inputs["x"], dtype=f4))
    w1h, bias2, wpk = _host_weights(
        np.asarray(inputs["Adj_t"], dtype=f4),
        np.asarray(inputs["Adj_s"], dtype=f4),
        np.asarray(inputs["s"], dtype=f4),
        np.asarray(inputs["H"], dtype=f4),
        np.asarray(inputs["W1"], dtype=f4),
        np.asarray(inputs["b1"], dtype=f4),
        np.asarray(inputs["W2"], dtype=f4),
        np.asarray(inputs["b2"], dtype=f4),
    )
    maps = []
    for c in range(NCORES):
        b, q = c // NQ, c % NQ
        maps.append(
            {
                "xb": np.ascontiguousarray(x[b]),
                "w1h": w1h,
                "bias2": bias2,
                "wpk": np.ascontiguousarray(wpk[q]),
            }
        )
    return maps


def kernel(**inputs) -> np.ndarray:
    from concourse import bass_utils

    if "nc" not in _CACHE:
        _CACHE["nc"] = _build_nc()
    nc = _CACHE["nc"]

    maps = _in_maps(inputs)
    import os

    trace = bool(int(os.environ.get("GTCNN_TRACE", "0")))
    res = bass_utils.run_bass_kernel_spmd(
        nc,
        maps,
        core_ids=list(range(NCORES)),
        trace=trace,
        trace_cores=list(range(NCORES)) if trace else None,
        stitch_traces=False,
    )
    _CACHE["last_results"] = res

    out = np.empty((B, M, FOUT), dtype=np.float32)
    for c in range(NCORES):
        b, q = c // NQ, c % NQ
        out[b, 2048 * q : 2048 * (q + 1), :] = res.results[c]["outb"]
    return out


# revision 26
# speedup vs baseline: 1.5145x; 1.0075x over previous
"""Trainium2 Bass kernel for nn_GTCNN (product-graph GTCNN, 2 layers, K collapsed).

Math (per batch b, x: [M=8192, 32]):
  Adj = s0*I + s1*kron(I_t, As) + s2*kron(At, I_s) + s3*kron(At, As),  T=64, N=128
  h0 = x @ W1 + b1
  h_{l+1} = tanh((Adj @ h_l) @ Heff_l),   Heff_l = sum_k H[l, k]
  out = h2 @ W2 + b2

Device algorithm (Heff commutes with Adj, so Heff1 folds into W1 host-side):
  w  = x @ (W1 Heff1) + 1 (x) (b1 Heff1)          [FD matmul]
  z1 = tanh(P w + Q At-mix(w))                     [layer 1, all t]
  y  = P z1[:, q] + Q At[q,:]-mix(z1)              [layer 2, t-quarter]
  z2 = tanh(y @ Heff2);  out = z2 @ W2 + b2
  with P = s0*I + s1*As, Q = s2*I + s3*As folded on host.

Sharding: core c -> (b = c // 4, t-quarter q = c % 4). Layer 1 computed fully per
b (4x redundant; collectives have a ~10us floor, far above the redundant work).

Layouts (n = 32*nh + nl, t = 32*c + tl):
  NM  [n, t*32 + h]                      node-on-partition (P / Q matmuls)
  FD  [32*nh + h,  t*32 + nl]            feature-on-partition (W/Heff stationaries
                                         are block-diag kron(I4, W))
  FDT [32*nh + tl, ...]                  t-on-partition (At matmuls, stationaries
                                         kron(I4, At 32x32 block), PSUM-accum c)
All matmuls bf16 (PSUM fp32); PSUM evictions are scalar-engine copies casting to
bf16; layout moves are DVE 32x32 StreamTranspose ops on bf16 SBUF tiles. Every
transpose WRITES with stride-1 within-block (strided DVE writes cost ~3.6x);
consuming matmuls absorb the resulting layout via strided moving-AP views.
Tiles that land transposed-conventions: u_nm col = 64h+32cp+tl, g2 col =
1024c+32h+nl, u2_nm col = 32h+i.
"""

import numpy as np

T, NS, B, FIN, HID, FOUT = 64, 128, 2, 32, 32, 16
M = T * NS
NCORES, NQ = 8, 4
TQ = T // NQ  # 16 t's per quarter

_CACHE = {}

# bf16 weight pack [128, 1408]; 128-col slot i: atbd[2c+cp] i=0..3, atbq[c]
# i=4..5, P i=6, Q i=7, hi4_2 i=8, w2i4 i=9, w1hi4 i=10.
WPK_COLS = 1408


def _build_nc():
    from contextlib import ExitStack

    import concourse.mybir as mybir
    import concourse.tile as tile
    from concourse import bacc
    from concourse.bass import ds

    fp = mybir.dt.float32
    bf = mybir.dt.bfloat16
    AF = mybir.ActivationFunctionType

    nc = bacc.Bacc(
        "TRN2",
        target_bir_lowering=False,
        debug=False,
        enable_asserts=False,
        num_devices=NCORES,
    )

    xb = nc.dram_tensor("xb", [M, FIN], fp, kind="ExternalInput")
    w1h = nc.dram_tensor("w1h", [128, 128], fp, kind="ExternalInput")
    bias2 = nc.dram_tensor("bias2", [128, 2], fp, kind="ExternalInput")
    wpk = nc.dram_tensor("wpk", [128, WPK_COLS], bf, kind="ExternalInput")
    outb = nc.dram_tensor("outb", [TQ * NS, FOUT], fp, kind="ExternalOutput")

    C512 = [slice(512 * j, 512 * (j + 1)) for j in range(4)]
    C1024 = [slice(1024 * j, 1024 * (j + 1)) for j in range(2)]

    with tile.TileContext(nc) as tc, ExitStack() as ctx:
        const = ctx.enter_context(tc.tile_pool(name="const", bufs=1))
        st = ctx.enter_context(tc.tile_pool(name="st", bufs=1))
        ps = ctx.enter_context(tc.tile_pool(name="ps", bufs=4, space="PSUM"))


        # ---- PE warm-up on an uninitialized tile: no input deps, so these
        # run at t~0 while DMAs stream, releasing the HAM clock-gate (PE is
        # ~2x slower until ~4us of sustained activity). Output never read.
        junk = const.tile([128, 512], bf, tag="junk")
        nc.gpsimd.memset(junk[:], 0)
        warm_ps = ps.tile([128, 512], fp, tag="big")
        for _ in range(17):
            nc.tensor.matmul(warm_ps[:], junk[:, 0:128], junk[:], start=True, stop=True)

        # ---- x load first (the critical path): NM [n, (t, f)], 4 t-chunks
        # split across the two HWDGE rings (sync + scalar queues).
        wpk_s = const.tile([128, WPK_COLS], bf, tag="wpk")
        nc.gpsimd.dma_start(wpk_s[:], wpk.ap())
        w1h_s = const.tile([128, 128], fp, tag="w1h")
        nc.gpsimd.dma_start(w1h_s[:], w1h.ap())
        bias_s = const.tile([128, 2], fp, tag="bias")
        nc.gpsimd.dma_start(bias_s[:], bias2.ap())

        x_nm = st.tile([128, 2048], fp, tag="x_nm")
        xv = xb.ap().rearrange("(t n) f -> n t f", n=128)
        x_nm_v = x_nm[:].rearrange("p (t f) -> p t f", f=32)
        x_engs = [nc.sync, nc.scalar, nc.sync, nc.scalar, nc.sync, nc.scalar, nc.gpsimd, nc.gpsimd]
        for j in range(8):
            x_engs[j].dma_start(x_nm_v[:, 8 * j : 8 * (j + 1), :], xv[:, 8 * j : 8 * (j + 1), :])
        wslot = wpk_s[:].rearrange("p (i c) -> p i c", c=128)
        pmat = wslot[:, 6, :]
        qmat = wslot[:, 7, :]
        hi4_2 = wslot[:, 8, :]
        w2i4 = wslot[:, 9, :]

        # ---- per chunk: NM -> FD (DVE, fp32; casting on ACT would put the
        # x-DMA issue queue in front of the casts and stall the ladder) ----
        x_fd = st.tile([128, 2048], fp, tag="x_fd")
        for j in range(8):
            nc.vector.transpose(out=x_fd[:, 256 * j : 256 * (j + 1)], in_=x_nm[:, 256 * j : 256 * (j + 1)])

        def pe_keepalive(k):
            # Dep-free LDWEIGHTS on the junk tile: occupies the otherwise-idle
            # PE between matmul stages so the HAM clock-gate stays released.
            for _ in range(k):
                nc.tensor.ldweights(junk[:, 0:128])

        # ---- w = x @ W1H + b1H  (FD, fp32 matmuls), ACT bias-evict bf16.
        # The whole layer-1 midsection is interleaved at t-half (c) granularity
        # so the PE never idles long enough to re-engage the HAM throttle:
        # evict[c] -> {w_nm[c], g1[c]} -> u1 mms for contraction-half c ->
        # P mms for output-half c, with the u eviction/transpose and Q mms
        # trailing one half behind.
        wpre_h = [ps.tile([128, 1024], fp, tag="big", name=f"wpre{c}") for c in range(2)]
        for j in range(4):
            nc.tensor.matmul(
                wpre_h[j // 2][:, 512 * (j % 2) : 512 * (j % 2) + 512],
                w1h_s[:],
                x_fd[:, C512[j]],
                start=True,
                stop=True,
            )
        pe_keepalive(16)

        w_fd = st.tile([128, 2048], bf, tag="w_fd")
        w_nm = st.tile([128, 2048], bf, tag="w_nm")
        g1 = st.tile([128, 2048], bf, tag="g1")
        u_ps_h = [ps.tile([128, 1024], fp, tag="big", name=f"ups{c}") for c in range(2)]
        gi = w_fd[:].rearrange("p (c tl nl) -> p c nl tl", c=2, tl=32, nl=32)
        go = g1[:].rearrange("p (c nl h) -> p c nl h", c=2, nl=32, h=32)
        g1m = g1[:].rearrange("p (c nl h) -> p c nl h", c=2, nl=32, h=32)
        zpre_h = [None, None]

        for c in range(2):
            nc.scalar.activation(
                w_fd[:, C1024[c]], wpre_h[c][:], AF.Identity, bias=bias_s[:, 0:1]
            )
            nc.vector.transpose(out=go[:, c], in_=gi[:, c])
            nc.vector.transpose(out=w_nm[:, C1024[c]], in_=w_fd[:, C1024[c]])
            # u1 contributions from contraction-half c (both output halves).
            # Moving view streams (nl, h-half) so the innermost stride is 1
            # (strided innermost moving reads run the PE ~1.7x slower).
            for cp in range(2):
                for hh in range(2):
                    nc.tensor.matmul(
                        u_ps_h[cp][:, 512 * hh : 512 * (hh + 1)],
                        wslot[:, 2 * c + cp, :],
                        g1m[:, c, :, 16 * hh : 16 * (hh + 1)],
                        start=(c == 0),
                        stop=(c == 1),
                    )
            # P mms for output chunks of this half (needs only w_nm[c])
            zpre_h[c] = ps.tile([128, 1024], fp, tag="big", name=f"zpre{c}")
            for j in (2 * c, 2 * c + 1):
                nc.tensor.matmul(
                    zpre_h[c][:, 512 * (j % 2) : 512 * (j % 2) + 512],
                    pmat,
                    w_nm[:, C512[j]],
                    start=True,
                    stop=False,
                )

        # ---- ACT-evict u (cast bf16), FDT -> NM on DVE (u_nm t-inner:
        # col = 64h + 32cp + tl, so the transpose writes stride-1) ----
        u_fdt = st.tile([128, 2048], bf, tag="u_fdt")
        u_nm = st.tile([128, 2048], bf, tag="u_nm")
        # u_fdt col = 1024cp + 512hh + 16nl + hlow ; u_nm col = 1024hh + 64hlow
        # + 32cp + tl (t-inner for the stride-1 transpose write).
        ui = u_fdt[:].rearrange("p (cp hh nl h) -> p cp hh h nl", cp=2, hh=2, nl=32, h=16)
        uo = u_nm[:].rearrange("p (hh h cp tl) -> p cp hh h tl", hh=2, h=16, cp=2, tl=32)
        u_mv = u_nm[:].rearrange("p (hh h cp tl) -> p cp hh h tl", hh=2, h=16, cp=2, tl=32)
        for cp in range(2):
            nc.scalar.activation(u_fdt[:, C1024[cp]], u_ps_h[cp][:], AF.Identity)
            nc.vector.transpose(out=uo[:, cp], in_=ui[:, cp])
            # Q mms: stream (h, tl) from a stride-1 moving view into a strided
            # PSUM out AP that still lands zpre cols as (tl, h).
            for k in range(2):
                zq_out = zpre_h[cp][:, 512 * k : 512 * (k + 1)].rearrange(
                    "p (tl h) -> p h tl", tl=16, h=32
                )
                zq_o4 = zq_out.rearrange("p (hh h) tl -> p hh h tl", hh=2, h=16)
                nc.tensor.matmul(
                    zq_o4,
                    qmat,
                    u_mv[:, cp, :, :, 16 * k : 16 * (k + 1)],
                    start=False,
                    stop=True,
                )

        z1_nm = st.tile([128, 2048], bf, tag="z1_nm")
        for j in range(2):
            nc.scalar.activation(z1_nm[:, C1024[j]], zpre_h[j][:], AF.Tanh)

        # GPSIMD (idle) extracts this core's t-quarter of z1 so the layer-2
        # P-matmul gets a register-free moving AP (register APs on the PE cost
        # ~1.7us in TENSOR_LOADs right on the layer-2 critical path).
        pidg = nc.gpsimd.partition_id()
        toffg = (pidg % NQ) * TQ
        z1v = z1_nm[:].rearrange("p (t h) -> p t h", h=32)
        zq_cp = st.tile([128, 512], bf, tag="zq_cp")
        nc.gpsimd.tensor_copy(zq_cp[:], z1v[:, ds(toffg, TQ), :])

        # ====================== layer 2 (t-quarter only) ======================
        # g2 = FDT'(z1), stored nl-inner: col = 1024c + 32h + nl.
        g2 = st.tile([128, 2048], bf, tag="g2")
        zi = z1_nm[:].rearrange("p (c tl h) -> p c h tl", c=2, tl=32, h=32)
        zo = g2[:].rearrange("p (c h nl) -> p c h nl", c=2, h=32, nl=32)
        for c in range(2):
            nc.vector.transpose(out=zo[:, c], in_=zi[:, c])

        # zpre2 P-part first: runs on the PE while the u2 path's evict and
        # transpose are still in flight.
        zpre2 = ps.tile([128, 512], fp, tag="big")
        nc.tensor.matmul(zpre2[:], pmat, zq_cp[:], start=True, stop=False)

        # u2 = At[q rows]-mix(z1): out partitions (nh, tl' in 0..15)
        u2_ps = ps.tile([128, 1024], fp, tag="big")
        g2r = g2[:].rearrange("p (c h nl) -> p c h nl", c=2, h=32, nl=32)
        for hh in range(2):
            for c in range(2):
                nc.tensor.matmul(
                    u2_ps[:, 512 * hh : 512 * (hh + 1)],
                    wslot[:, 4 + c, :],
                    g2r[:, c, 16 * hh : 16 * (hh + 1), :],
                    start=(c == 0),
                    stop=(c == 1),
                )

        u2_f = st.tile([128, 1024], bf, tag="u2_f")
        nc.scalar.activation(u2_f[:], u2_ps[:], AF.Identity)

        # u2_nm stored i-inner (col = 32h + i), one 1024-el transpose.
        u2_nm = st.tile([128, 1024], bf, tag="u2_nm")
        u2i = u2_f[:].rearrange("p (h nl) -> p h nl", h=32, nl=32)
        u2o = u2_nm[:].rearrange("p (h i) -> p h i", h=32, i=32)
        nc.vector.transpose(out=u2o[:], in_=u2i[:])

        # zpre2 = P zq + Q u2 (NM quarter), evict bf16
        u2_mv = u2_nm[:].rearrange("p (h i) -> p h i", h=32, i=32)
        z2_out = zpre2[:].rearrange("p (tq h) -> p h tq", tq=16, h=32)
        nc.tensor.matmul(z2_out, qmat, u2_mv[:, :, 0:16], start=False, stop=True)

        zq_nm = st.tile([128, 512], bf, tag="zq_nm")
        nc.scalar.activation(zq_nm[:], zpre2[:], AF.Identity)

        # NM -> FD, Heff2 matmul + tanh, W2 matmul + bias, FD -> NM, DMA out
        zq_fd = st.tile([128, 512], bf, tag="zq_fd")
        nc.vector.transpose(out=zq_fd[:], in_=zq_nm[:])

        pre2 = ps.tile([128, 512], fp, tag="big")
        nc.tensor.matmul(pre2[:], hi4_2, zq_fd[:], start=True, stop=True)
        h2_fd = st.tile([128, 512], bf, tag="h2_fd")
        nc.scalar.activation(h2_fd[:], pre2[:], AF.Tanh)

        opre = ps.tile([128, 512], fp, tag="big")
        nc.tensor.matmul(opre[:], w2i4, h2_fd[:], start=True, stop=True)
        out_fd = st.tile([128, 512], fp, tag="out_fd")
        nc.scalar.activation(out_fd[:], opre[:], AF.Identity, bias=bias_s[:, 1:2])

        out_nm = st.tile([128, 512], fp, tag="out_nm")
        nc.vector.transpose(out=out_nm[:], in_=out_fd[:])

        onv = out_nm[:].rearrange("p (i j2) -> p i j2", j2=32)
        ov = outb.ap().rearrange("(i n) j -> n i j", n=128)
        nc.sync.dma_start(ov, onv[:, :, 0:FOUT])

    nc.compile()
    return nc


def _host_weights(Adj_t, Adj_s, s, H, W1, b1, W2, b2):
    import ml_dtypes

    f4 = np.float32
    bf = ml_dtypes.bfloat16
    I4 = np.eye(4, dtype=f4)
    I128 = np.eye(128, dtype=f4)
    Heff = H.sum(axis=1).astype(f4)  # [2, 32, 32]

    P = (s[0] * I128 + s[1] * Adj_s).astype(f4)
    Q = (s[2] * I128 + s[3] * Adj_s).astype(f4)

    W1H = (W1 @ Heff[0]).astype(f4)
    b1H = (b1 @ Heff[0]).astype(f4)

    hi4_2 = np.kron(I4, Heff[1])
    w2pad = np.zeros((32, 32), dtype=f4)
    w2pad[:, :FOUT] = W2
    w2i4 = np.kron(I4, w2pad)

    bias2 = np.zeros((128, 2), dtype=f4)
    bias2[:, 0] = np.tile(b1H, 4)
    b2pad = np.zeros(32, dtype=f4)
    b2pad[:FOUT] = b2
    bias2[:, 1] = np.tile(b2pad, 4)

    wpk = np.zeros((NQ, 128, WPK_COLS), dtype=bf)
    for c in range(2):
        for cp in range(2):
            blk = np.kron(I4, Adj_t[32 * c : 32 * (c + 1), 32 * cp : 32 * (cp + 1)].astype(f4))
            wpk[:, :, 128 * (2 * c + cp) : 128 * (2 * c + cp + 1)] = blk.astype(bf)
    for q in range(NQ):
        for c in range(2):
            blk = np.zeros((32, 32), dtype=f4)
            blk[:, :TQ] = Adj_t[32 * c : 32 * (c + 1), TQ * q : TQ * (q + 1)]
            wpk[q, :, 128 * (4 + c) : 128 * (5 + c)] = np.kron(I4, blk).astype(bf)
    wpk[:, :, 128 * 6 : 128 * 7] = P.astype(bf)
    wpk[:, :, 128 * 7 : 128 * 8] = Q.astype(bf)
    wpk[:, :, 128 * 8 : 128 * 9] = hi4_2.astype(bf)
    wpk[:, :, 128 * 9 : 128 * 10] = w2i4.astype(bf)
    w1h = np.kron(I4, W1H)

    return w1h, bias2, wpk


def _in_maps(inputs):
    f4 = np.float32
    x = np.ascontiguousarray(np.asarray(inputs["x"], dtype=f4))
    w1h, bias2, wpk = _host_weights(
        np.asarray(inputs["Adj_t"], dtype=f4),
        np.asarray(inputs["Adj_s"], dtype=f4),
        np.asarray(inputs["s"], dtype=f4),
        np.asarray(inputs["H"], dtype=f4),
        np.asarray(inputs["W1"], dtype=f4),
        np.asarray(inputs["b1"], dtype=f4),
        np.asarray(inputs["W2"], dtype=f4),
        np.asarray(inputs["b2"], dtype=f4),
    )
    maps = []
    for c in range(NCORES):
        b, q = c // NQ, c % NQ
        maps.append(
            {
                "xb": np.ascontiguousarray(x[b]),
                "w1h": w1h,
                "bias2": bias2,
                "wpk": np.ascontiguousarray(wpk[q]),
            }
        )
    return maps


def kernel(**inputs) -> np.ndarray:
    from concourse import bass_utils

    if "nc" not in _CACHE:
        _CACHE["nc"] = _build_nc()
    nc = _CACHE["nc"]

    maps = _in_maps(inputs)
    import os

    trace = bool(int(os.environ.get("GTCNN_TRACE", "0")))
    res = bass_utils.run_bass_kernel_spmd(
        nc,
        maps,
        core_ids=list(range(NCORES)),
        trace=trace,
        trace_cores=list(range(NCORES)) if trace else None,
        stitch_traces=False,
    )
    _CACHE["last_results"] = res

    out = np.empty((B, M, FOUT), dtype=np.float32)
    for c in range(NCORES):
        b, q = c // NQ, c % NQ
        out[b, 2048 * q : 2048 * (q + 1), :] = res.results[c]["outb"]
    return out


# revision 28
# speedup vs baseline: 1.6025x; 1.0580x over previous
"""Trainium2 Bass kernel for nn_GTCNN (product-graph GTCNN, 2 layers, K collapsed).

Math (per batch b, x: [M=8192, 32]):
  Adj = s0*I + s1*kron(I_t, As) + s2*kron(At, I_s) + s3*kron(At, As),  T=64, N=128
  h0 = x @ W1 + b1
  h_{l+1} = tanh((Adj @ h_l) @ Heff_l),   Heff_l = sum_k H[l, k]
  out = h2 @ W2 + b2

Device algorithm (Heff commutes with Adj, so Heff1 folds into W1 host-side):
  w  = x @ (W1 Heff1) + 1 (x) (b1 Heff1)          [FD matmul]
  z1 = tanh(P w + Q At-mix(w))                     [layer 1, all t]
  y  = P z1[:, q] + Q At[q,:]-mix(z1)              [layer 2, t-quarter]
  z2 = tanh(y @ Heff2);  out = z2 @ W2 + b2
  with P = s0*I + s1*As, Q = s2*I + s3*As folded on host.

Sharding: core c -> (b = c // 4, t-quarter q = c % 4). Layer 1 computed fully per
b (4x redundant; collectives have a ~10us floor, far above the redundant work).

Layouts (n = 32*nh + nl, t = 32*c + tl):
  NM  [n, t*32 + h]                      node-on-partition (P / Q matmuls)
  FD  [32*nh + h,  t*32 + nl]            feature-on-partition (W/Heff stationaries
                                         are block-diag kron(I4, W))
  FDT [32*nh + tl, ...]                  t-on-partition (At matmuls, stationaries
                                         kron(I4, At 32x32 block), PSUM-accum c)
All matmuls bf16 (PSUM fp32); PSUM evictions are scalar-engine copies casting to
bf16; layout moves are DVE 32x32 StreamTranspose ops on bf16 SBUF tiles. Every
transpose WRITES with stride-1 within-block (strided DVE writes cost ~3.6x);
consuming matmuls absorb the resulting layout via strided moving-AP views.
Tiles that land transposed-conventions: u_nm col = 64h+32cp+tl, g2 col =
1024c+32h+nl, u2_nm col = 32h+i.
"""

import numpy as np

T, NS, B, FIN, HID, FOUT = 64, 128, 2, 32, 32, 16
M = T * NS
NCORES, NQ = 8, 4
TQ = T // NQ  # 16 t's per quarter

_CACHE = {}

# bf16 weight pack [128, 1408]; 128-col slot i: atbd[2c+cp] i=0..3, atbq[c]
# i=4..5, P i=6, Q i=7, hi4_2 i=8, w2i4 i=9, w1hi4 i=10.
WPK_COLS = 1408


def _build_nc():
    from contextlib import ExitStack

    import concourse.mybir as mybir
    import concourse.tile as tile
    from concourse import bacc
    from concourse.bass import ds

    fp = mybir.dt.float32
    bf = mybir.dt.bfloat16
    AF = mybir.ActivationFunctionType

    nc = bacc.Bacc(
        "TRN2",
        target_bir_lowering=False,
        debug=False,
        enable_asserts=False,
        num_devices=NCORES,
    )

    xb = nc.dram_tensor("xb", [M, FIN], fp, kind="ExternalInput")
    w1h = nc.dram_tensor("w1h", [128, 128], fp, kind="ExternalInput")
    bias2 = nc.dram_tensor("bias2", [128, 2], fp, kind="ExternalInput")
    wpk = nc.dram_tensor("wpk", [128, WPK_COLS], bf, kind="ExternalInput")
    outb = nc.dram_tensor("outb", [TQ * NS, FOUT], fp, kind="ExternalOutput")

    C512 = [slice(512 * j, 512 * (j + 1)) for j in range(4)]
    C1024 = [slice(1024 * j, 1024 * (j + 1)) for j in range(2)]

    with tile.TileContext(nc) as tc, ExitStack() as ctx:
        const = ctx.enter_context(tc.tile_pool(name="const", bufs=1))
        st = ctx.enter_context(tc.tile_pool(name="st", bufs=1))
        ps = ctx.enter_context(tc.tile_pool(name="ps", bufs=4, space="PSUM"))


        # ---- PE warm-up on an uninitialized tile: no input deps, so these
        # run at t~0 while DMAs stream, releasing the HAM clock-gate (PE is
        # ~2x slower until ~4us of sustained activity). Output never read.
        junk = const.tile([128, 512], bf, tag="junk")
        nc.gpsimd.memset(junk[:], 0)
        warm_ps = ps.tile([128, 512], fp, tag="big")
        for _ in range(20):
            nc.tensor.matmul(warm_ps[:], junk[:, 0:128], junk[:], start=True, stop=True)

        # ---- x load first (the critical path): NM [n, (t, f)], 4 t-chunks
        # split across the two HWDGE rings (sync + scalar queues).
        wpk_s = const.tile([128, WPK_COLS], bf, tag="wpk")
        nc.gpsimd.dma_start(wpk_s[:], wpk.ap())
        w1h_s = const.tile([128, 128], fp, tag="w1h")
        nc.gpsimd.dma_start(w1h_s[:], w1h.ap())
        bias_s = const.tile([128, 2], fp, tag="bias")
        nc.gpsimd.dma_start(bias_s[:], bias2.ap())

        x_nm = st.tile([128, 2048], fp, tag="x_nm")
        xv = xb.ap().rearrange("(t n) f -> n t f", n=128)
        x_nm_v = x_nm[:].rearrange("p (t f) -> p t f", f=32)
        x_engs = [nc.sync, nc.scalar, nc.sync, nc.scalar, nc.sync, nc.scalar, nc.gpsimd, nc.gpsimd]
        for j in range(8):
            x_engs[j].dma_start(x_nm_v[:, 8 * j : 8 * (j + 1), :], xv[:, 8 * j : 8 * (j + 1), :])
        wslot = wpk_s[:].rearrange("p (i c) -> p i c", c=128)
        pmat = wslot[:, 6, :]
        qmat = wslot[:, 7, :]
        hi4_2 = wslot[:, 8, :]
        w2i4 = wslot[:, 9, :]

        # ---- per chunk: NM -> FD (DVE, fp32; casting on ACT would put the
        # x-DMA issue queue in front of the casts and stall the ladder) ----
        x_fd = st.tile([128, 2048], fp, tag="x_fd")
        for j in range(8):
            nc.vector.transpose(out=x_fd[:, 256 * j : 256 * (j + 1)], in_=x_nm[:, 256 * j : 256 * (j + 1)])

        def pe_keepalive(k):
            # Dep-free LDWEIGHTS on the junk tile: occupies the otherwise-idle
            # PE between matmul stages so the HAM clock-gate stays released.
            for _ in range(k):
                nc.tensor.ldweights(junk[:, 0:128])

        # ---- w = x @ W1H + b1H  (FD, fp32 matmuls), ACT bias-evict bf16.
        # The whole layer-1 midsection is interleaved at t-half (c) granularity
        # so the PE never idles long enough to re-engage the HAM throttle:
        # evict[c] -> {w_nm[c], g1[c]} -> u1 mms for contraction-half c ->
        # P mms for output-half c, with the u eviction/transpose and Q mms
        # trailing one half behind.
        wpre_h = [ps.tile([128, 1024], fp, tag="big", name=f"wpre{c}") for c in range(2)]
        for j in range(4):
            nc.tensor.matmul(
                wpre_h[j // 2][:, 512 * (j % 2) : 512 * (j % 2) + 512],
                w1h_s[:],
                x_fd[:, C512[j]],
                start=True,
                stop=True,
            )
        pe_keepalive(16)

        w_fd = st.tile([128, 2048], bf, tag="w_fd")
        w_nm = st.tile([128, 2048], bf, tag="w_nm")
        g1 = st.tile([128, 2048], bf, tag="g1")
        u_ps_h = [ps.tile([128, 1024], fp, tag="big", name=f"ups{c}") for c in range(2)]
        gi = w_fd[:].rearrange("p (c tl nl) -> p c nl tl", c=2, tl=32, nl=32)
        go = g1[:].rearrange("p (c nl h) -> p c nl h", c=2, nl=32, h=32)
        g1m = g1[:].rearrange("p (c nl h) -> p c nl h", c=2, nl=32, h=32)
        zpre_h = [None, None]

        for c in range(2):
            nc.scalar.activation(
                w_fd[:, C1024[c]], wpre_h[c][:], AF.Identity, bias=bias_s[:, 0:1]
            )
            nc.vector.transpose(out=go[:, c], in_=gi[:, c])
            nc.vector.transpose(out=w_nm[:, C1024[c]], in_=w_fd[:, C1024[c]])
            # u1 contributions from contraction-half c (both output halves).
            # Moving view streams (nl, h-half) so the innermost stride is 1
            # (strided innermost moving reads run the PE ~1.7x slower).
            for cp in range(2):
                for hh in range(2):
                    nc.tensor.matmul(
                        u_ps_h[cp][:, 512 * hh : 512 * (hh + 1)],
                        wslot[:, 2 * c + cp, :],
                        g1m[:, c, :, 16 * hh : 16 * (hh + 1)],
                        start=(c == 0),
                        stop=(c == 1),
                    )
            # P mms for output chunks of this half (needs only w_nm[c])
            zpre_h[c] = ps.tile([128, 1024], fp, tag="big", name=f"zpre{c}")
            for j in (2 * c, 2 * c + 1):
                nc.tensor.matmul(
                    zpre_h[c][:, 512 * (j % 2) : 512 * (j % 2) + 512],
                    pmat,
                    w_nm[:, C512[j]],
                    start=True,
                    stop=False,
                )

        # ---- ACT-evict u (cast bf16), FDT -> NM on DVE (u_nm t-inner:
        # col = 64h + 32cp + tl, so the transpose writes stride-1) ----
        u_fdt = st.tile([128, 2048], bf, tag="u_fdt")
        u_nm = st.tile([128, 2048], bf, tag="u_nm")
        # u_fdt col = 1024cp + 512hh + 16nl + hlow ; u_nm col = 1024hh + 64hlow
        # + 32cp + tl (t-inner for the stride-1 transpose write).
        ui = u_fdt[:].rearrange("p (cp hh nl h) -> p cp hh h nl", cp=2, hh=2, nl=32, h=16)
        uo = u_nm[:].rearrange("p (hh h cp tl) -> p cp hh h tl", hh=2, h=16, cp=2, tl=32)
        u_mv = u_nm[:].rearrange("p (hh h cp tl) -> p cp hh h tl", hh=2, h=16, cp=2, tl=32)
        for cp in range(2):
            nc.scalar.activation(u_fdt[:, C1024[cp]], u_ps_h[cp][:], AF.Identity)
            nc.vector.transpose(out=uo[:, cp], in_=ui[:, cp])
            # Q mms: stream (h, tl) from a stride-1 moving view into a strided
            # PSUM out AP that still lands zpre cols as (tl, h).
            for k in range(2):
                zq_out = zpre_h[cp][:, 512 * k : 512 * (k + 1)].rearrange(
                    "p (tl h) -> p h tl", tl=16, h=32
                )
                zq_o4 = zq_out.rearrange("p (hh h) tl -> p hh h tl", hh=2, h=16)
                nc.tensor.matmul(
                    zq_o4,
                    qmat,
                    u_mv[:, cp, :, :, 16 * k : 16 * (k + 1)],
                    start=False,
                    stop=True,
                )

        z1_nm = st.tile([128, 2048], bf, tag="z1_nm")
        for j in range(2):
            nc.scalar.activation(z1_nm[:, C1024[j]], zpre_h[j][:], AF.Tanh)

        # GPSIMD (idle) extracts this core's t-quarter of z1 so the layer-2
        # P-matmul gets a register-free moving AP (register APs on the PE cost
        # ~1.7us in TENSOR_LOADs right on the layer-2 critical path).
        pidg = nc.gpsimd.partition_id()
        toffg = (pidg % NQ) * TQ
        z1v = z1_nm[:].rearrange("p (t h) -> p t h", h=32)
        zq_cp = st.tile([128, 512], bf, tag="zq_cp")
        nc.gpsimd.tensor_copy(zq_cp[:], z1v[:, ds(toffg, TQ), :])

        # ====================== layer 2 (t-quarter only) ======================
        # g2 = FDT'(z1), stored nl-inner: col = 1024c + 32h + nl.
        g2 = st.tile([128, 2048], bf, tag="g2")
        zi = z1_nm[:].rearrange("p (c tl h) -> p c h tl", c=2, tl=32, h=32)
        zo = g2[:].rearrange("p (c h nl) -> p c h nl", c=2, h=32, nl=32)
        for c in range(2):
            nc.vector.transpose(out=zo[:, c], in_=zi[:, c])

        # zpre2 P-part first: runs on the PE while the u2 path's evict and
        # transpose are still in flight.
        zpre2 = ps.tile([128, 512], fp, tag="big")
        nc.tensor.matmul(zpre2[:], pmat, zq_cp[:], start=True, stop=False)

        # u2 = At[q rows]-mix(z1): out partitions (nh, tl' in 0..15)
        u2_ps = ps.tile([128, 1024], fp, tag="big")
        g2r = g2[:].rearrange("p (c h nl) -> p c h nl", c=2, h=32, nl=32)
        for hh in range(2):
            for c in range(2):
                nc.tensor.matmul(
                    u2_ps[:, 512 * hh : 512 * (hh + 1)],
                    wslot[:, 4 + c, :],
                    g2r[:, c, 16 * hh : 16 * (hh + 1), :],
                    start=(c == 0),
                    stop=(c == 1),
                )

        u2_f = st.tile([128, 1024], bf, tag="u2_f")
        nc.scalar.activation(u2_f[:], u2_ps[:], AF.Identity)

        # u2_nm stored i-inner (col = 32h + i), one 1024-el transpose.
        u2_nm = st.tile([128, 1024], bf, tag="u2_nm")
        u2i = u2_f[:].rearrange("p (h nl) -> p h nl", h=32, nl=32)
        u2o = u2_nm[:].rearrange("p (h i) -> p h i", h=32, i=32)
        nc.vector.transpose(out=u2o[:], in_=u2i[:])

        # zpre2 = P zq + Q u2 (NM quarter), evict bf16
        u2_mv = u2_nm[:].rearrange("p (h i) -> p h i", h=32, i=32)
        z2_out = zpre2[:].rearrange("p (tq h) -> p h tq", tq=16, h=32)
        nc.tensor.matmul(z2_out, qmat, u2_mv[:, :, 0:16], start=False, stop=True)

        zq_nm = st.tile([128, 512], bf, tag="zq_nm")
        nc.scalar.activation(zq_nm[:], zpre2[:], AF.Identity)

        # NM -> FD, Heff2 matmul + tanh, W2 matmul + bias, FD -> NM, DMA out
        zq_fd = st.tile([128, 512], bf, tag="zq_fd")
        nc.vector.transpose(out=zq_fd[:], in_=zq_nm[:])

        pre2 = ps.tile([128, 512], fp, tag="big")
        nc.tensor.matmul(pre2[:], hi4_2, zq_fd[:], start=True, stop=True)
        h2_fd = st.tile([128, 512], bf, tag="h2_fd")
        nc.scalar.activation(h2_fd[:], pre2[:], AF.Tanh)

        opre = ps.tile([128, 512], fp, tag="big")
        nc.tensor.matmul(opre[:], w2i4, h2_fd[:], start=True, stop=True)
        out_fd = st.tile([128, 512], fp, tag="out_fd")
        nc.scalar.activation(out_fd[:], opre[:], AF.Identity, bias=bias_s[:, 1:2])

        out_nm = st.tile([128, 512], fp, tag="out_nm")
        onv = out_nm[:].rearrange("p (i j2) -> p i j2", j2=32)
        ov = outb.ap().rearrange("(i n) j -> n i j", n=128)
        for k in range(2):
            nc.vector.transpose(
                out=out_nm[:, 256 * k : 256 * (k + 1)], in_=out_fd[:, 256 * k : 256 * (k + 1)]
            )
            eng = nc.sync if k == 0 else nc.scalar
            eng.dma_start(ov[:, 8 * k : 8 * (k + 1), :], onv[:, 8 * k : 8 * (k + 1), 0:FOUT])

    nc.compile()
    return nc


def _host_weights(Adj_t, Adj_s, s, H, W1, b1, W2, b2):
    import ml_dtypes

    f4 = np.float32
    bf = ml_dtypes.bfloat16
    I4 = np.eye(4, dtype=f4)
    I128 = np.eye(128, dtype=f4)
    Heff = H.sum(axis=1).astype(f4)  # [2, 32, 32]

    P = (s[0] * I128 + s[1] * Adj_s).astype(f4)
    Q = (s[2] * I128 + s[3] * Adj_s).astype(f4)

    W1H = (W1 @ Heff[0]).astype(f4)
    b1H = (b1 @ Heff[0]).astype(f4)

    hi4_2 = np.kron(I4, Heff[1])
    w2pad = np.zeros((32, 32), dtype=f4)
    w2pad[:, :FOUT] = W2
    w2i4 = np.kron(I4, w2pad)

    bias2 = np.zeros((128, 2), dtype=f4)
    bias2[:, 0] = np.tile(b1H, 4)
    b2pad = np.zeros(32, dtype=f4)
    b2pad[:FOUT] = b2
    bias2[:, 1] = np.tile(b2pad, 4)

    wpk = np.zeros((NQ, 128, WPK_COLS), dtype=bf)
    for c in range(2):
        for cp in range(2):
            blk = np.kron(I4, Adj_t[32 * c : 32 * (c + 1), 32 * cp : 32 * (cp + 1)].astype(f4))
            wpk[:, :, 128 * (2 * c + cp) : 128 * (2 * c + cp + 1)] = blk.astype(bf)
    for q in range(NQ):
        for c in range(2):
            blk = np.zeros((32, 32), dtype=f4)
            blk[:, :TQ] = Adj_t[32 * c : 32 * (c + 1), TQ * q : TQ * (q + 1)]
            wpk[q, :, 128 * (4 + c) : 128 * (5 + c)] = np.kron(I4, blk).astype(bf)
    wpk[:, :, 128 * 6 : 128 * 7] = P.astype(bf)
    wpk[:, :, 128 * 7 : 128 * 8] = Q.astype(bf)
    wpk[:, :, 128 * 8 : 128 * 9] = hi4_2.astype(bf)
    wpk[:, :, 128 * 9 : 128 * 10] = w2i4.astype(bf)
    w1h = np.kron(I4, W1H)

    return w1h, bias2, wpk


def _in_maps(inputs):
    f4 = np.float32
    x = np.ascontiguousarray(np.asarray(inputs["x"], dtype=f4))
    w1h, bias2, wpk = _host_weights(
        np.asarray(inputs["Adj_t"], dtype=f4),
        np.asarray(inputs["Adj_s"], dtype=f4),
        np.asarray(inputs["s"], dtype=f4),
        np.asarray(inputs["H"], dtype=f4),
        np.asarray(inputs["W1"], dtype=f4),
        np.asarray(inputs["b1"], dtype=f4),
        np.asarray(inputs["W2"], dtype=f4),
        np.asarray(inputs["b2"], dtype=f4),
    )
    maps = []
    for c in range(NCORES):
        b, q = c // NQ, c % NQ
        maps.append(
            {
                "xb": np.ascontiguousarray(x[b]),
                "w1h": w1h,
                "bias2": bias2,
                "wpk": np.ascontiguousarray(wpk[q]),
            }
        )
    return maps


def kernel(**inputs) -> np.ndarray:
    from concourse import bass_utils

    if "nc" not in _CACHE:
        _CACHE["nc"] = _build_nc()
    nc = _CACHE["nc"]

    maps = _in_maps(inputs)
    import os

    trace = bool(int(os.environ.get("GTCNN_TRACE", "0")))
    res = bass_utils.run_bass_kernel_spmd(
        nc,
        maps,
        core_ids=list(range(NCORES)),
        trace=trace,
        trace_cores=list(range(NCORES)) if trace else None,
        stitch_traces=False,
    )
    _CACHE["last_results"] = res

    out = np.empty((B, M, FOUT), dtype=np.float32)
    for c in range(NCORES):
        b, q = c // NQ, c % NQ
        out[b, 2048 * q : 2048 * (q + 1), :] = res.results[c]["outb"]
    return out
